# revision 13
# baseline (speedup 1.0000x reference)
"""Trainium2 Bass kernel for nn_CompleteAttention_68418829025814.

Linformer-style windowed attention, restructured for the PE array:
  - window_reverse is folded into a host-side column permutation of E_w/F_w
    (device works entirely in x's native window order) and a host-side
    permutation of the gathered output.
  - k/v are never materialized: k_low = (E @ x) @ Wk^T + const (the E/F
    projections contract over tokens, so x is used in its native layout).
  - q path is fp8 DoubleRow with the qkv bias folded in via an augmented
    contract row (ones in x, bias in W); the fp8 2^7 gain is undone by
    scaling k_low by 2^-7 host-side so the q PSUM->SBUF move is a plain copy.
  - per-tile pipeline: q proj (one tile ahead) -> scores (row-band packed,
    one single-bank PSUM tile per head; multi-bank PSUM tensors are broken
    on HW) -> exp per head -> attn@V + denominators (col-band packed) ->
    reciprocal+divide on DVE -> output projection with proj_w stationary
    (output feature-major [c_out, token]; final transpose on host).
  - all PSUM tiles are single-bank and rotate through one 8-buf pool; the
    proj outputs alias the avA/zA banks after they are consumed.

Sharding: data-parallel over batch; each of the 8 cores gets 4 batches
(256 windows) of x. Small weights are replicated.
"""

import numpy as np

B_TOT = 32
N_CORES = 8
B_PER = B_TOT // N_CORES      # 4 batches per core
N = 3136                      # tokens per batch
NP = 3200                     # padded tokens per batch (6*512 + 128)
C = 192
H = 6
HD = 32
R = 128
WS = 7

_STATE = {}


def _window_perm():
    """n_of_m[m] = spatial index n for window-order position m."""
    hh, ww, i, j = np.meshgrid(
        np.arange(8), np.arange(8), np.arange(7), np.arange(7), indexing="ij"
    )
    m = (hh * 8 + ww) * 49 + i * 7 + j
    n = (hh * 7 + i) * 56 + ww * 7 + j
    n_of_m = np.empty(N, dtype=np.int64)
    n_of_m[m.ravel()] = n.ravel()
    return n_of_m


def _build_bass():
    import concourse.bacc as bacc
    import concourse.mybir as mybir
    from concourse.tile import TileContext

    f32 = mybir.dt.float32
    f16 = mybir.dt.float16
    f8 = mybir.dt.float8e4

    nc = bacc.Bacc("TRN2", target_bir_lowering=False, debug=False)

    # x_a: phase-A layout, row (p2*NP + n) = [x[2*p2, n, :], x[2*p2+1, n, :]]
    x_d = nc.dram_tensor("x_a", [2 * NP, 2 * C], f16, kind="ExternalInput")
    # xT for the q projection, fp8, contract 192+bias split as [97, 2] rows
    # (DoubleRow layout): row k<96, half p, col j = x[j, 96*p + k];
    # row 96 = [ones, zeros] (bias row).
    xq_d = nc.dram_tensor("xq8", [97, 2 * B_PER * NP], f8, kind="ExternalInput")
    wq8h_d = nc.dram_tensor("wq8_hi", [97, 2 * 128], f8, kind="ExternalInput")
    wq8l_d = nc.dram_tensor("wq8_lo", [97, 2 * 64], f8, kind="ExternalInput")
    # e/f shipped pre-chunked: row p = 24 chunks of 128 R-values (token 128k+p)
    e_d = nc.dram_tensor("e_wxt", [128, 24 * R], f16, kind="ExternalInput")
    f_d = nc.dram_tensor("f_wxt", [128, 24 * R], f16, kind="ExternalInput")
    e_tl_d = nc.dram_tensor("e_tl", [64, R], f16, kind="ExternalInput")
    f_tl_d = nc.dram_tensor("f_tl", [64, R], f16, kind="ExternalInput")
    wkt_d = nc.dram_tensor("wkt", [C, C], f16, kind="ExternalInput")   # * 2^-7
    wvt_d = nc.dram_tensor("wvt", [C, C], f16, kind="ExternalInput")
    ckt_d = nc.dram_tensor("const_kt", [C, R], f32, kind="ExternalInput")  # * 2^-7
    cv_d = nc.dram_tensor("const_v", [R, C], f32, kind="ExternalInput")
    # proj weights, stationary chunks: pw = proj_w.T (ch, co)
    pwhh_d = nc.dram_tensor("pw_hh", [128, 128], f16, kind="ExternalInput")
    pwhl_d = nc.dram_tensor("pw_hl", [128, 64], f16, kind="ExternalInput")
    pwlh_d = nc.dram_tensor("pw_lh_aug", [65, 128], f16, kind="ExternalInput")
    pwll_d = nc.dram_tensor("pw_ll_aug", [65, 64], f16, kind="ExternalInput")
    ident_d = nc.dram_tensor("ident", [128, 128], f16, kind="ExternalInput")
    ones_d = nc.dram_tensor("ones_att", [128, 32], f16, kind="ExternalInput")
    onesrow_d = nc.dram_tensor("ones_row", [1, 512], f16, kind="ExternalInput")
    # outputs feature-major: out[c, b*NP + n]
    outh_d = nc.dram_tensor("out_hi", [128, B_PER * NP], f16, kind="ExternalOutput")
    outl_d = nc.dram_tensor("out_lo", [64, B_PER * NP], f16, kind="ExternalOutput")
    DEBUG = _STATE.get("debug", False)
    if DEBUG:
        dbg_klo = nc.dram_tensor("dbg_klo", [128, R], f16, kind="ExternalOutput")
        dbg_vlo = nc.dram_tensor("dbg_vlo", [128, C], f16, kind="ExternalOutput")
        dbg_qsb = nc.dram_tensor("dbg_qsb", [128, 1024], f16, kind="ExternalOutput")
        dbg_sp = nc.dram_tensor("dbg_sp", [128, 512], f16, kind="ExternalOutput")
        dbg_avhi = nc.dram_tensor("dbg_avhi", [128, 512], f16, kind="ExternalOutput")
        dbg_avlo = nc.dram_tensor("dbg_avlo", [65, 512], f16, kind="ExternalOutput")

    NCH = 25  # n-chunks per batch for the E/F contraction (24*128 + 64)

    with TileContext(nc) as tc:
        with tc.tile_pool(name="const", bufs=1) as cpool, \
             tc.tile_pool(name="ef", bufs=1) as efpool, \
             tc.tile_pool(name="low", bufs=1) as lowpool, \
             tc.tile_pool(name="xin", bufs=6) as xpool, \
             tc.tile_pool(name="xq", bufs=3) as xqpool, \
             tc.tile_pool(name="qs", bufs=9) as qpool, \
             tc.tile_pool(name="sp", bufs=2) as sppool, \
             tc.tile_pool(name="div", bufs=2) as divpool, \
             tc.tile_pool(name="av", bufs=2) as avpool, \
             tc.tile_pool(name="osb", bufs=2) as opool, \
             tc.tile_pool(name="ps", bufs=8, space="PSUM") as ps:

            # ---- constants ----
            ident = cpool.tile([128, 128], f16)
            nc.scalar.dma_start(ident[:], ident_d[:])
            wq8h = cpool.tile([97, 2, 128], f8)
            nc.scalar.dma_start(
                wq8h[:], wq8h_d[:].rearrange("p (two m) -> p two m", two=2)
            )
            wq8l = cpool.tile([97, 2, 64], f8)
            nc.scalar.dma_start(
                wq8l[:], wq8l_d[:].rearrange("p (two m) -> p two m", two=2)
            )
            wkt = cpool.tile([128, C], f16)
            nc.scalar.dma_start(wkt[:], wkt_d[0:128, :])
            wkt_l = cpool.tile([64, C], f16)
            nc.scalar.dma_start(wkt_l[:], wkt_d[128:192, :])
            wvt = cpool.tile([128, C], f16)
            nc.scalar.dma_start(wvt[:], wvt_d[0:128, :])
            wvt_l = cpool.tile([64, C], f16)
            nc.scalar.dma_start(wvt_l[:], wvt_d[128:192, :])
            ckt_h = cpool.tile([128, R], f32)
            nc.scalar.dma_start(ckt_h[:], ckt_d[0:128, :])
            ckt_l = cpool.tile([64, R], f32)
            nc.scalar.dma_start(ckt_l[:], ckt_d[128:192, :])
            cv = cpool.tile([128, C], f32)
            nc.scalar.dma_start(cv[:], cv_d[:])
            pw_hh = cpool.tile([128, 128], f16)
            nc.scalar.dma_start(pw_hh[:], pwhh_d[:])
            pw_hl = cpool.tile([128, 64], f16)
            nc.scalar.dma_start(pw_hl[:], pwhl_d[:])
            pw_lh = cpool.tile([65, 128], f16)
            nc.scalar.dma_start(pw_lh[:], pwlh_d[:])
            pw_ll = cpool.tile([65, 64], f16)
            nc.scalar.dma_start(pw_ll[:], pwll_d[:])
            ones_att = cpool.tile([128, 32], f16)
            nc.scalar.dma_start(ones_att[:], ones_d[:])
            # persistent [65, 512] attn-output staging tiles whose row 64
            # stays 1.0 forever (feeds proj_b through pw_l*_aug's last row)
            av_lo_bufs = [cpool.tile([65, 512], f16, name=f"avlo{i}") for i in range(2)]
            for i in range(2):
                nc.scalar.dma_start(av_lo_bufs[i][64:65, :], onesrow_d[:])

            # E/F transposed weights resident in SBUF: 24 full chunks + tail
            e_sb = efpool.tile([128, 24, 128], f16)
            f_sb = efpool.tile([128, 24, 128], f16)
            e_tl = efpool.tile([64, 128], f16)
            f_tl = efpool.tile([64, 128], f16)

            def load_ef_group(g):
                sl = slice(g * 6, (g + 1) * 6)
                dsl = slice(g * 6 * 128, (g + 1) * 6 * 128)
                nc.sync.dma_start(
                    e_sb[:, sl, :].rearrange("p k r -> p (k r)"), e_d[:, dsl]
                )
                nc.sync.dma_start(
                    f_sb[:, sl, :].rearrange("p k r -> p (k r)"), f_d[:, dsl]
                )

            load_ef_group(0)

            # per-batch low-rank tensors (kept resident across phase B)
            klo_h = [lowpool.tile([128, R], f16, name=f"klo_h{b}") for b in range(B_PER)]
            klo_l = [lowpool.tile([64, R], f16, name=f"klo_l{b}") for b in range(B_PER)]
            vlo = [lowpool.tile([128, C], f16, name=f"vlo{b}") for b in range(B_PER)]

            xq_v = xq_d[:].rearrange("p (two m) -> p two m", two=2)

            # q projection for one (b, t) tile -> [128, 2W] f16 (hi | lo).
            # fp8 DoubleRow; bias rides contract row 96, gain undone in klo.
            def qproj(b, t):
                W = 512 if t < 6 else 128
                base = b * NP + t * 512
                xq = xqpool.tile([97, 2, W], f8, name="xq", tag="xq")
                nc.sync.dma_start(xq[:], xq_v[:, :, base : base + W])
                qh_ps = ps.tile([128, W], f32, name="qh_ps", tag="bank")
                nc.tensor.matmul(
                    qh_ps[:], wq8h[:], xq[:], start=True, stop=True,
                    perf_mode=mybir.MatmulPerfMode.DoubleRow,
                )
                ql_ps = ps.tile([64, W], f32, name="ql_ps", tag="bank")
                nc.tensor.matmul(
                    ql_ps[:], wq8l[:], xq[:], start=True, stop=True,
                    perf_mode=mybir.MatmulPerfMode.DoubleRow,
                )
                qsb = qpool.tile([128, 2 * W], f16, name="qsb", tag="qsb")
                nc.vector.tensor_copy(qsb[:, 0:W], qh_ps[:])
                nc.vector.tensor_copy(qsb[0:64, W : 2 * W], ql_ps[:])
                return qsb

            # ---------------- Phase A: EP/FP + low-rank projections ----------
            # q tiles for batch 0 are computed inside the EP/FP stream so the
            # PE stays dense during x2 DMA gaps and phase B can start at once.
            q_b0 = [None] * 7
            for p2 in range(2):
                ep_ps = ps.tile([128, 2 * C], f32, name="ep_ps", tag="bank")
                fp_ps = ps.tile([128, 2 * C], f32, name="fp_ps", tag="bank")
                for ci in range(NCH):
                    nk = 128 if ci < 24 else 64
                    x2 = xpool.tile([nk, 2 * C], f16, name="x2", tag="x2")
                    nc.sync.dma_start(
                        x2[:],
                        x_d[p2 * NP + ci * 128 : p2 * NP + ci * 128 + nk, :],
                    )
                    if p2 == 0 and ci in (1, 6, 11):
                        load_ef_group(ci // 5 + 1)
                    if p2 == 0 and ci == 16:
                        nc.sync.dma_start(e_tl[:], e_tl_d[:])
                        nc.sync.dma_start(f_tl[:], f_tl_d[:])
                    if ci % 4 == 1 and (p2 * NCH + ci) // 4 < 7:
                        ti = (p2 * NCH + ci) // 4
                        q_b0[ti] = qproj(0, ti)
                        if DEBUG and ti == 0:
                            nc.sync.dma_start(dbg_qsb[:, 0:512], q_b0[0][:, 0:512])
                            nc.sync.dma_start(
                                dbg_qsb[0:64, 512:1024], q_b0[0][0:64, 512:1024]
                            )
                    elh = e_sb[:, ci, :] if ci < 24 else e_tl[:]
                    flh = f_sb[:, ci, :] if ci < 24 else f_tl[:]
                    x2f = x2[:]
                    nc.tensor.matmul(
                        ep_ps[:], elh, x2f, start=(ci == 0), stop=(ci == NCH - 1)
                    )
                    nc.tensor.matmul(
                        fp_ps[:], flh, x2f, start=(ci == 0), stop=(ci == NCH - 1)
                    )
                ep_sb = xpool.tile([128, 2 * C], f16, name="ep_sb", tag="ep_sb")
                nc.vector.tensor_copy(ep_sb[:], ep_ps[:])
                fp_sb = xpool.tile([128, 2 * C], f16, name="fp_sb", tag="fp_sb")
                nc.vector.tensor_copy(fp_sb[:], fp_ps[:])

                for b2 in range(2):
                    b = 2 * p2 + b2
                    # transpose EP, FP slices: (r=128, c=192) -> (c, r)
                    ept_h = xpool.tile([128, 128], f16, name="ept_h", tag="ept_h")
                    ept_l = xpool.tile([64, 128], f16, name="ept_l", tag="ept_l")
                    fpt_h = xpool.tile([128, 128], f16, name="fpt_h", tag="fpt_h")
                    fpt_l = xpool.tile([64, 128], f16, name="fpt_l", tag="fpt_l")
                    for (src, dsth, dstl) in ((ep_sb, ept_h, ept_l), (fp_sb, fpt_h, fpt_l)):
                        tp1 = ps.tile([128, 128], f16, name="tp1", tag="bank")
                        nc.tensor.transpose(
                            tp1[:], src[:, b2 * C : b2 * C + 128], ident[:]
                        )
                        nc.vector.tensor_copy(dsth[:], tp1[:])
                        tp2 = ps.tile([64, 128], f16, name="tp2", tag="bank")
                        nc.tensor.transpose(
                            tp2[:], src[:, b2 * C + 128 : b2 * C + 192], ident[:]
                        )
                        nc.vector.tensor_copy(dstl[:], tp2[:])

                    # k_lowT = WkT.T @ EPT + const_kT  (feature-major (kch, r))
                    kl_hi = ps.tile([128, R], f32, name="kl_hi", tag="bank")
                    nc.tensor.matmul(kl_hi[:], wkt[:, 0:128], ept_h[:], start=True, stop=False)
                    nc.tensor.matmul(kl_hi[:], wkt_l[:, 0:128], ept_l[:], start=False, stop=True)
                    nc.vector.tensor_tensor(
                        klo_h[b][:], kl_hi[:], ckt_h[:], op=mybir.AluOpType.add
                    )
                    kl_lo = ps.tile([64, R], f32, name="kl_lo", tag="bank")
                    nc.tensor.matmul(kl_lo[:], wkt[:, 128:192], ept_h[:], start=True, stop=False)
                    nc.tensor.matmul(kl_lo[:], wkt_l[:, 128:192], ept_l[:], start=False, stop=True)
                    nc.vector.tensor_tensor(
                        klo_l[b][:], kl_lo[:], ckt_l[:], op=mybir.AluOpType.add
                    )
                    # v_low (R-major (r, vch)), straight to f16 with const add
                    vl_ps = ps.tile([128, C], f32, name="vl_ps", tag="bank")
                    nc.tensor.matmul(vl_ps[:], fpt_h[:], wvt[:], start=True, stop=False)
                    nc.tensor.matmul(vl_ps[:], fpt_l[:], wvt_l[:], start=False, stop=True)
                    nc.vector.tensor_tensor(
                        vlo[b][:], vl_ps[:], cv[:], op=mybir.AluOpType.add
                    )
                    if DEBUG and b == 0:
                        nc.sync.dma_start(dbg_klo[:], klo_h[0][:])
                        nc.sync.dma_start(dbg_vlo[:], vlo[0][:])

            # ---------------- Phase B: attention tiles ----------
            def front(b, t, qsb):
                W = 512 if t < 6 else 128
                base = b * NP + t * 512
                # scores: one single-bank psum tile per head, row-band packed
                sps = []
                for h in range(6):
                    s1 = ps.tile([128, W], f32, name=f"s{h}", tag="bank")
                    if h < 4:
                        nc.tensor.matmul(
                            s1[:],
                            klo_h[b][32 * h : 32 * h + 32, :],
                            qsb[32 * h : 32 * h + 32, 0:W],
                            start=True, stop=True,
                            tile_position=(32 * h, 0),
                        )
                    else:
                        hh = h - 4
                        nc.tensor.matmul(
                            s1[:],
                            klo_l[b][32 * hh : 32 * hh + 32, :],
                            qsb[32 * hh : 32 * hh + 32, W : 2 * W],
                            start=True, stop=True,
                            tile_position=(32 * hh, 0),
                        )
                    sp1 = sppool.tile([128, W], f16, name=f"sp{h}", tag=f"sp{h}")
                    nc.scalar.activation(
                        sp1[:], s1[:], mybir.ActivationFunctionType.Exp
                    )
                    if DEBUG and b == 0 and t == 0 and h == 0:
                        nc.sync.dma_start(dbg_sp[:], sp1[:])
                    sps.append(sp1)

                def spt(h):
                    return sps[h][:]

                # attn @ v_low + denominators, col-band packed
                avA = ps.tile([128, W], f32, name="avA", tag="bank")
                for h in range(4):
                    nc.tensor.matmul(
                        avA[32 * h : 32 * h + 32, :],
                        vlo[b][:, 32 * h : 32 * h + 32],
                        spt(h),
                        start=True, stop=True,
                        tile_position=(0, 32 * h),
                    )
                zA = ps.tile([128, W], f32, name="zA", tag="bank")
                for h in range(4):
                    nc.tensor.matmul(
                        zA[32 * h : 32 * h + 32, :],
                        ones_att[:],
                        spt(h),
                        start=True, stop=True,
                        tile_position=(0, 32 * h),
                    )
                # lo heads (DVE PSUM reads must start at partition 0, so av2
                # and z2 get their own tiles)
                av2 = ps.tile([64, W], f32, name="av2", tag="bank")
                for hh in range(2):
                    nc.tensor.matmul(
                        av2[32 * hh : 32 * hh + 32, :],
                        vlo[b][:, 128 + 32 * hh : 160 + 32 * hh],
                        spt(4 + hh),
                        start=True, stop=True,
                        tile_position=(0, 32 * hh),
                    )
                z2 = ps.tile([64, W], f32, name="z2", tag="bank")
                for hh in range(2):
                    nc.tensor.matmul(
                        z2[32 * hh : 32 * hh + 32, :],
                        ones_att[:],
                        spt(4 + hh),
                        start=True, stop=True,
                        tile_position=(0, 32 * hh),
                    )
                av_hi = avpool.tile([128, W], f16, name="av_hi", tag="av_hi")
                av_lo = av_lo_bufs[(b * 7 + t) % 2]
                return dict(
                    W=W, base=base, t=t, avA=avA, zA=zA, av2=av2, z2=z2,
                    av_hi=av_hi, av_lo=av_lo,
                )

            osb_state = {}

            def back(st):
                W, base, t = st["W"], st["base"], st["t"]
                rzA = divpool.tile([128, W], f32, name="rzA", tag="rzA")
                nc.vector.reciprocal_approx_fast(rzA[:], st["zA"][:])
                rz2 = divpool.tile([64, W], f32, name="rz2", tag="rz2")
                nc.vector.reciprocal_approx_fast(rz2[:], st["z2"][:])
                av_hi, av_lo = st["av_hi"], st["av_lo"]
                nc.vector.tensor_tensor(
                    av_hi[:, :], st["avA"][:], rzA[:], op=mybir.AluOpType.mult
                )
                nc.vector.tensor_tensor(
                    av_lo[0:64, 0:W], st["av2"][:], rz2[:], op=mybir.AluOpType.mult
                )
                if DEBUG and base == 0:
                    nc.sync.dma_start(dbg_avhi[:, 0:W], av_hi[:, 0:W])
                    nc.sync.dma_start(dbg_avlo[:, 0:W], av_lo[:, 0:W])
                # output projection: proj weights stationary, av moving; the
                # psum outputs alias the (already consumed) avA / zA banks.
                o1 = st["avA"]
                nc.tensor.matmul(o1[:], pw_hh[:], av_hi[:, 0:W], start=True, stop=False)
                nc.tensor.matmul(o1[:], pw_lh[:], av_lo[:, 0:W], start=False, stop=True)
                o2 = st["zA"][0:64, :]
                nc.tensor.matmul(o2, pw_hl[:], av_hi[:, 0:W], start=True, stop=False)
                nc.tensor.matmul(o2, pw_ll[:], av_lo[:, 0:W], start=False, stop=True)
                # stage pairs of tiles and DMA once per pair
                if t % 2 == 0:
                    osb_state["hi"] = opool.tile([128, 1024], f16, name="osb", tag="osb")
                    osb_state["lo"] = opool.tile([64, 1024], f16, name="osb2", tag="osb2")
                    osb_state["base"] = base
                off = (t % 2) * 512
                osb, osb2 = osb_state["hi"], osb_state["lo"]
                nc.scalar.copy(osb[:, off : off + W], o1[:])
                nc.vector.tensor_copy(osb2[:, off : off + W], o2)
                if t % 2 == 1 or t == 6:
                    w_tot = 512 + W if t % 2 == 1 else W
                    b0 = osb_state["base"]
                    nc.gpsimd.dma_start(
                        outh_d[:, b0 : b0 + w_tot], osb[:, 0:w_tot]
                    )
                    nc.gpsimd.dma_start(
                        outl_d[:, b0 : b0 + w_tot], osb2[:, 0:w_tot]
                    )

            tiles = [(b, t) for b in range(B_PER) for t in range(7)]
            prev = None
            qsb_cur = q_b0[0]
            for i, (b, t) in enumerate(tiles):
                st = front(b, t, qsb_cur)
                if i + 1 < len(tiles):
                    bn, tn = tiles[i + 1]
                    qsb_cur = q_b0[tn] if bn == 0 else qproj(bn, tn)
                if prev is not None:
                    back(prev)
                prev = st
            back(prev)

    nc.compile()
    return nc


def _get_nc():
    if "nc" not in _STATE:
        _STATE["nc"] = _build_bass()
    return _STATE["nc"]


def kernel(x, qkv_w, qkv_b, E_w, E_b, F_w, F_b, proj_w, proj_b, h, w):
    from concourse.bass_utils import run_bass_kernel_spmd

    x = np.asarray(x, dtype=np.float32)
    qkv_w = np.asarray(qkv_w, dtype=np.float32)
    qkv_b = np.asarray(qkv_b, dtype=np.float32)
    E_w = np.asarray(E_w, dtype=np.float32)
    E_b = np.asarray(E_b, dtype=np.float32)
    F_w = np.asarray(F_w, dtype=np.float32)
    F_b = np.asarray(F_b, dtype=np.float32)
    proj_w = np.asarray(proj_w, dtype=np.float32)
    proj_b = np.asarray(proj_b, dtype=np.float32)
    assert int(h) == 56 and int(w) == 56

    n_of_m = _window_perm()
    E_wx = np.ascontiguousarray(E_w[:, n_of_m])
    F_wx = np.ascontiguousarray(F_w[:, n_of_m])

    Wq, Wk, Wv = qkv_w[0:C], qkv_w[C : 2 * C], qkv_w[2 * C : 3 * C]
    bq, bk, bv = qkv_b[0:C], qkv_b[C : 2 * C], qkv_b[2 * C : 3 * C]
    scale = np.float32(1.0 / np.sqrt(HD))

    const_k = np.outer(E_wx.sum(1), bk) + E_b[:, None]      # (128, 192)
    const_v = (np.outer(F_wx.sum(1), bv) + F_b[:, None]).astype(np.float32)

    import ml_dtypes

    f8np = ml_dtypes.float8_e4m3
    # q weights in fp8 DoubleRow layout [97, 2, m] with a 2^7 gain; row 96
    # carries the (scaled, gained) bias, paired with a ones-row in xq8.
    wqs_t = np.ascontiguousarray((Wq * scale).T * 128.0)      # (c, m)
    wq_dr = np.zeros((97, 2, C), dtype=np.float32)
    wq_dr[0:96] = wqs_t.reshape(2, 96, C).transpose(1, 0, 2)
    wq_dr[96, 0] = bq * scale * 128.0
    wq8_hi = np.ascontiguousarray(wq_dr[:, :, 0:128]).reshape(97, 256).astype(f8np)
    wq8_lo = np.ascontiguousarray(wq_dr[:, :, 128:192]).reshape(97, 128).astype(f8np)
    # 2^-7 gain undo baked into the k_low path
    wkt = np.ascontiguousarray(Wk.T * (2.0 ** -7)).astype(np.float16)
    wvt = np.ascontiguousarray(Wv.T).astype(np.float16)
    ckt = np.ascontiguousarray(const_k.T.astype(np.float32) * (2.0 ** -7))  # (192, 128)
    pw = proj_w.T                                            # (ch, co)
    pw_hh = np.ascontiguousarray(pw[0:128, 0:128]).astype(np.float16)
    pw_hl = np.ascontiguousarray(pw[0:128, 128:192]).astype(np.float16)
    pw_lh = np.zeros((65, 128), dtype=np.float16)
    pw_lh[0:64] = pw[128:192, 0:128]
    pw_lh[64] = proj_b[0:128]
    pw_ll = np.zeros((65, 64), dtype=np.float16)
    pw_ll[0:64] = pw[128:192, 128:192]
    pw_ll[64] = proj_b[128:192]

    e_wxt_full = np.ascontiguousarray(E_wx.T).astype(np.float16)  # (3136, 128)
    f_wxt_full = np.ascontiguousarray(F_wx.T).astype(np.float16)
    # pre-chunked layout: (24, 128, R) -> (128, 24*R) so each SBUF partition
    # line is one contiguous DMA descriptor
    e_wxt = np.ascontiguousarray(
        e_wxt_full[0:3072].reshape(24, 128, R).transpose(1, 0, 2).reshape(128, 24 * R)
    )
    f_wxt = np.ascontiguousarray(
        f_wxt_full[0:3072].reshape(24, 128, R).transpose(1, 0, 2).reshape(128, 24 * R)
    )
    e_tl = np.ascontiguousarray(e_wxt_full[3072:3136])
    f_tl = np.ascontiguousarray(f_wxt_full[3072:3136])
    ident = np.eye(128, dtype=np.float16)
    ones_att = np.ones((128, 32), dtype=np.float16)
    ones_row = np.ones((1, 512), dtype=np.float16)

    consts = dict(
        e_wxt=e_wxt, f_wxt=f_wxt, e_tl=e_tl, f_tl=f_tl,
        wq8_hi=wq8_hi, wq8_lo=wq8_lo, wkt=wkt, wvt=wvt,
        const_kt=ckt, const_v=const_v,
        pw_hh=pw_hh, pw_hl=pw_hl, pw_lh_aug=pw_lh, pw_ll_aug=pw_ll,
        ident=ident, ones_att=ones_att, ones_row=ones_row,
    )

    # shard x: core i gets batches 4i..4i+4, padded to NP tokens per batch
    xb = x.reshape(B_TOT, 64 * 49, C).astype(np.float16)
    in_maps = []
    for i in range(N_CORES):
        xi = np.zeros((B_PER, NP, C), dtype=np.float16)
        xi[:, 0:N, :] = xb[B_PER * i : B_PER * (i + 1)]
        # phase-A layout: (p2, n, pair*C) so chunk loads are fully contiguous
        xa = np.ascontiguousarray(
            xi.reshape(2, 2, NP, C).transpose(0, 2, 1, 3).reshape(2 * NP, 2 * C)
        )
        M = B_PER * NP
        xt = xi.reshape(M, C).T                               # (c, M)
        xq8 = np.zeros((97, 2, M), dtype=np.float32)
        xq8[0:96] = xt.reshape(2, 96, M).transpose(1, 0, 2)
        xq8[96, 0] = 1.0
        xq8 = np.ascontiguousarray(xq8.reshape(97, 2 * M)).astype(f8np)
        in_maps.append({**consts, "x_a": xa, "xq8": xq8})

    nc = _get_nc()
    _STATE["last_in_maps"] = in_maps
    res = run_bass_kernel_spmd(nc, in_maps, core_ids=list(range(N_CORES)))

    out_win = np.empty((B_TOT, N, C), dtype=np.float32)
    for i in range(N_CORES):
        oh = res.results[i]["out_hi"].astype(np.float32)      # (128, B_PER*NP)
        ol = res.results[i]["out_lo"].astype(np.float32)      # (64, B_PER*NP)
        oc = np.concatenate([oh, ol], axis=0)                 # (192, B_PER*NP)
        oi = oc.reshape(C, B_PER, NP).transpose(1, 2, 0)      # (B_PER, NP, C)
        out_win[B_PER * i : B_PER * (i + 1)] = oi[:, 0:N, :]
    # window_reverse on the gathered output
    out_sp = (
        out_win.reshape(B_TOT, 8, 8, 7, 7, C)
        .transpose(0, 1, 3, 2, 4, 5)
        .reshape(B_TOT, N, C)
    )
    return np.ascontiguousarray(out_sp)


# revision 15
# speedup vs baseline: 1.1635x; 1.1635x over previous
"""Trainium2 Bass kernel for nn_CompleteAttention_68418829025814.

Linformer-style windowed attention, restructured for the PE array:
  - window_reverse is folded into a host-side column permutation of E_w/F_w
    (device works entirely in x's native window order) and a host-side
    permutation of the gathered output.
  - k/v are never materialized: k_low = (E @ x) @ Wk^T + const (the E/F
    projections contract over tokens, so x is used in its native layout).
  - q path is fp8 DoubleRow with the qkv bias folded in via an augmented
    contract row (ones in x, bias in W); the fp8 2^7 gain is undone by
    scaling k_low by 2^-7 host-side so the q PSUM->SBUF move is a plain copy.
  - phase A contracts all 4 batches at once ([128, 768] x-chunks, double
    chunks per DMA on the otherwise-idle scalar queue), so k_low/v_low for
    every batch are ready ~25us in and phase B covers the rest.
  - phase B per-tile emission is ordered for the in-order engine queues:
    scores+exps(t) -> back(t-1) (recip/divide/proj) -> qproj(t+1) -> av/z(t),
    keeping the PE fed (proj of t-1, q of t+1) while ACT runs t's exps.
  - output projection runs with proj_w stationary; outputs land feature-major
    [c_out, token] (psum banks aliased onto the consumed avA/zA tiles) and
    the final transpose happens on host.
  - HW constraints found the hard way: PSUM tensors must be single-bank, and
    engine reads of PSUM must start at partition 0.

Sharding: data-parallel over batch; each of the 8 cores gets 4 batches
(256 windows) of x. Small weights are replicated.
"""

import numpy as np

B_TOT = 32
N_CORES = 8
B_PER = B_TOT // N_CORES      # 4 batches per core
N = 3136                      # tokens per batch
NP = 3200                     # padded tokens per batch (6*512 + 128)
C = 192
H = 6
HD = 32
R = 128
WS = 7

_STATE = {}


def _window_perm():
    """n_of_m[m] = spatial index n for window-order position m."""
    hh, ww, i, j = np.meshgrid(
        np.arange(8), np.arange(8), np.arange(7), np.arange(7), indexing="ij"
    )
    m = (hh * 8 + ww) * 49 + i * 7 + j
    n = (hh * 7 + i) * 56 + ww * 7 + j
    n_of_m = np.empty(N, dtype=np.int64)
    n_of_m[m.ravel()] = n.ravel()
    return n_of_m


def _build_bass():
    import concourse.bacc as bacc
    import concourse.mybir as mybir
    from concourse.tile import TileContext

    f32 = mybir.dt.float32
    f16 = mybir.dt.float16
    f8 = mybir.dt.float8e4

    nc = bacc.Bacc("TRN2", target_bir_lowering=False, debug=False)

    # x_a: phase-A layout, row n = [x[0, n, :] | x[1, n, :] | x[2, n, :] | x[3, n, :]]
    x_d = nc.dram_tensor("x_a", [NP, 4 * C], f16, kind="ExternalInput")
    # xT for the q projection, fp8, contract 192+bias split as [97, 2] rows
    # (DoubleRow layout): row k<96, half p, col j = x[j, 96*p + k];
    # row 96 = [ones, zeros] (bias row).
    xq_d = nc.dram_tensor("xq8", [97, 2 * B_PER * NP], f8, kind="ExternalInput")
    wq8h_d = nc.dram_tensor("wq8_hi", [97, 2 * 128], f8, kind="ExternalInput")
    wq8l_d = nc.dram_tensor("wq8_lo", [97, 2 * 64], f8, kind="ExternalInput")
    # e/f shipped pre-chunked: row p = 24 chunks of 128 R-values (token 128k+p)
    e_d = nc.dram_tensor("e_wxt", [128, 24 * R], f16, kind="ExternalInput")
    f_d = nc.dram_tensor("f_wxt", [128, 24 * R], f16, kind="ExternalInput")
    e_tl_d = nc.dram_tensor("e_tl", [64, R], f16, kind="ExternalInput")
    f_tl_d = nc.dram_tensor("f_tl", [64, R], f16, kind="ExternalInput")
    wkt_d = nc.dram_tensor("wkt", [C, C], f16, kind="ExternalInput")   # * 2^-7
    wvt_d = nc.dram_tensor("wvt", [C, C], f16, kind="ExternalInput")
    ckt_d = nc.dram_tensor("const_kt", [C, R], f32, kind="ExternalInput")  # * 2^-7
    cv_d = nc.dram_tensor("const_v", [R, C], f32, kind="ExternalInput")
    # proj weights, stationary chunks: pw = proj_w.T (ch, co)
    pwhh_d = nc.dram_tensor("pw_hh", [128, 128], f16, kind="ExternalInput")
    pwhl_d = nc.dram_tensor("pw_hl", [128, 64], f16, kind="ExternalInput")
    pwlh_d = nc.dram_tensor("pw_lh_aug", [65, 128], f16, kind="ExternalInput")
    pwll_d = nc.dram_tensor("pw_ll_aug", [65, 64], f16, kind="ExternalInput")
    ident_d = nc.dram_tensor("ident", [128, 128], f16, kind="ExternalInput")
    ones_d = nc.dram_tensor("ones_att", [128, 32], f16, kind="ExternalInput")
    onesrow_d = nc.dram_tensor("ones_row", [1, 512], f16, kind="ExternalInput")
    # outputs feature-major: out[c, b*NP + n]
    outh_d = nc.dram_tensor("out_hi", [128, B_PER * NP], f16, kind="ExternalOutput")
    outl_d = nc.dram_tensor("out_lo", [64, B_PER * NP], f16, kind="ExternalOutput")

    NCH = 25  # n-chunks per batch for the E/F contraction (24*128 + 64)

    with TileContext(nc) as tc:
        with tc.tile_pool(name="const", bufs=1) as cpool, \
             tc.tile_pool(name="ef", bufs=1) as efpool, \
             tc.tile_pool(name="low", bufs=1) as lowpool, \
             tc.tile_pool(name="xin", bufs=4) as xpool, \
             tc.tile_pool(name="xq", bufs=3) as xqpool, \
             tc.tile_pool(name="qs", bufs=4) as qpool, \
             tc.tile_pool(name="sp", bufs=2) as sppool, \
             tc.tile_pool(name="div", bufs=2) as divpool, \
             tc.tile_pool(name="av", bufs=2) as avpool, \
             tc.tile_pool(name="osb", bufs=2) as opool, \
             tc.tile_pool(name="ps", bufs=4, space="PSUM") as ps:

            # ---- constants (scalar queue; phase A also runs on it) ----
            ident = cpool.tile([128, 128], f16)
            nc.scalar.dma_start(ident[:], ident_d[:])
            wq8h = cpool.tile([97, 2, 128], f8)
            nc.sync.dma_start(
                wq8h[:], wq8h_d[:].rearrange("p (two m) -> p two m", two=2)
            )
            wq8l = cpool.tile([97, 2, 64], f8)
            nc.sync.dma_start(
                wq8l[:], wq8l_d[:].rearrange("p (two m) -> p two m", two=2)
            )
            wkt = cpool.tile([128, C], f16)
            nc.scalar.dma_start(wkt[:], wkt_d[0:128, :])
            wkt_l = cpool.tile([64, C], f16)
            nc.scalar.dma_start(wkt_l[:], wkt_d[128:192, :])
            wvt = cpool.tile([128, C], f16)
            nc.scalar.dma_start(wvt[:], wvt_d[0:128, :])
            wvt_l = cpool.tile([64, C], f16)
            nc.scalar.dma_start(wvt_l[:], wvt_d[128:192, :])
            ckt_h = cpool.tile([128, R], f32)
            nc.scalar.dma_start(ckt_h[:], ckt_d[0:128, :])
            ckt_l = cpool.tile([64, R], f32)
            nc.scalar.dma_start(ckt_l[:], ckt_d[128:192, :])
            cv = cpool.tile([128, C], f32)
            nc.scalar.dma_start(cv[:], cv_d[:])
            pw_hh = cpool.tile([128, 128], f16)
            nc.sync.dma_start(pw_hh[:], pwhh_d[:])
            pw_hl = cpool.tile([128, 64], f16)
            nc.sync.dma_start(pw_hl[:], pwhl_d[:])
            pw_lh = cpool.tile([65, 128], f16)
            nc.sync.dma_start(pw_lh[:], pwlh_d[:])
            pw_ll = cpool.tile([65, 64], f16)
            nc.sync.dma_start(pw_ll[:], pwll_d[:])
            ones_att = cpool.tile([128, 32], f16)
            nc.sync.dma_start(ones_att[:], ones_d[:])
            # persistent [65, 512] attn-output staging tiles whose row 64
            # stays 1.0 forever (feeds proj_b through pw_l*_aug's last row)
            av_lo_bufs = [cpool.tile([65, 512], f16, name=f"avlo{i}") for i in range(2)]
            for i in range(2):
                nc.sync.dma_start(av_lo_bufs[i][64:65, :], onesrow_d[:])

            # E/F transposed weights resident in SBUF: 24 full chunks + tail
            e_sb = efpool.tile([128, 24, 128], f16)
            f_sb = efpool.tile([128, 24, 128], f16)
            e_tl = efpool.tile([64, 128], f16)
            f_tl = efpool.tile([64, 128], f16)

            def load_ef_group(g):
                sl = slice(g * 6, (g + 1) * 6)
                dsl = slice(g * 6 * 128, (g + 1) * 6 * 128)
                nc.scalar.dma_start(
                    e_sb[:, sl, :].rearrange("p k r -> p (k r)"), e_d[:, dsl]
                )
                nc.scalar.dma_start(
                    f_sb[:, sl, :].rearrange("p k r -> p (k r)"), f_d[:, dsl]
                )

            load_ef_group(0)

            # per-batch low-rank tensors (kept resident across phase B)
            klo_h = [lowpool.tile([128, R], f16, name=f"klo_h{b}") for b in range(B_PER)]
            klo_l = [lowpool.tile([64, R], f16, name=f"klo_l{b}") for b in range(B_PER)]
            vlo = [lowpool.tile([128, C], f16, name=f"vlo{b}") for b in range(B_PER)]

            xq_v = xq_d[:].rearrange("p (two m) -> p two m", two=2)

            # q projection for one (b, t) tile -> [128, 2W] f16 (hi | lo).
            # fp8 DoubleRow; bias rides contract row 96, gain undone in klo.
            def qproj(b, t):
                W = 512 if t < 6 else 128
                base = b * NP + t * 512
                xq = xqpool.tile([97, 2, W], f8, name="xq", tag="xq")
                nc.sync.dma_start(xq[:], xq_v[:, :, base : base + W])
                qh_ps = ps.tile([128, W], f32, name="qh_ps", tag="bk", bufs=4)
                nc.tensor.matmul(
                    qh_ps[:], wq8h[:], xq[:], start=True, stop=True,
                    perf_mode=mybir.MatmulPerfMode.DoubleRow,
                )
                ql_ps = ps.tile([64, W], f32, name="ql_ps", tag="bk", bufs=4)
                nc.tensor.matmul(
                    ql_ps[:], wq8l[:], xq[:], start=True, stop=True,
                    perf_mode=mybir.MatmulPerfMode.DoubleRow,
                )
                qsb = qpool.tile([128, 2 * W], f16, name="qsb", tag="qsb")
                nc.vector.tensor_copy(qsb[:, 0:W], qh_ps[:])
                nc.vector.tensor_copy(qsb[0:64, W : 2 * W], ql_ps[:])
                return qsb

            # ---------------- Phase A: EP/FP for all 4 batches ----------------
            # x chunks are [nk, 768] = 4 batches wide; two chunks per DMA.
            ep_a = ps.tile([128, 2 * C], f32, name="ep_a", tag="bk", bufs=4)
            ep_b = ps.tile([128, 2 * C], f32, name="ep_b", tag="bk", bufs=4)
            fp_a = ps.tile([128, 2 * C], f32, name="fp_a", tag="bk", bufs=4)
            fp_b = ps.tile([128, 2 * C], f32, name="fp_b", tag="bk", bufs=4)
            x2_tiles = {}
            for cp in range(13):
                k = 2 if cp < 12 else 1
                x2 = xpool.tile([128, 2, 4 * C], f16, name="x2", tag="x2")
                nc.scalar.dma_start(
                    x2[:, 0:k, :],
                    x_d[cp * 256 : cp * 256 + k * 128, :].rearrange(
                        "(k p) c -> p k c", p=128
                    ),
                )
                x2_tiles[cp] = x2
                if cp in (1, 4, 7):
                    load_ef_group(cp // 3 + 1)
                if cp == 10:
                    nc.scalar.dma_start(e_tl[:], e_tl_d[:])
                    nc.scalar.dma_start(f_tl[:], f_tl_d[:])
                for kk in range(k):
                    ci = cp * 2 + kk
                    nk = 128 if ci < 24 else 64
                    elh = e_sb[:, ci, :] if ci < 24 else e_tl[:]
                    flh = f_sb[:, ci, :] if ci < 24 else f_tl[:]
                    x2f = x2[0:nk, kk, :]
                    st = ci == 0
                    sp_ = ci == NCH - 1
                    nc.tensor.matmul(
                        ep_a[:], elh, x2f[:, 0 : 2 * C], start=st, stop=sp_
                    )
                    nc.tensor.matmul(
                        fp_a[:], flh, x2f[:, 0 : 2 * C], start=st, stop=sp_
                    )
                    nc.tensor.matmul(
                        ep_b[:], elh, x2f[:, 2 * C : 4 * C], start=st, stop=sp_
                    )
                    nc.tensor.matmul(
                        fp_b[:], flh, x2f[:, 2 * C : 4 * C], start=st, stop=sp_
                    )
            ep_sb = xpool.tile([128, 4 * C], f16, name="ep_sb", tag="ep_sb")
            nc.vector.tensor_copy(ep_sb[:, 0 : 2 * C], ep_a[:])
            nc.vector.tensor_copy(ep_sb[:, 2 * C : 4 * C], ep_b[:])
            fp_sb = xpool.tile([128, 4 * C], f16, name="fp_sb", tag="fp_sb")
            nc.vector.tensor_copy(fp_sb[:, 0 : 2 * C], fp_a[:])
            nc.vector.tensor_copy(fp_sb[:, 2 * C : 4 * C], fp_b[:])

            for b in range(B_PER):
                # transpose EP, FP slices: (r=128, c=192) -> (c, r)
                ept_h = xpool.tile([128, 128], f16, name="ept_h", tag="ept_h")
                ept_l = xpool.tile([64, 128], f16, name="ept_l", tag="ept_l")
                fpt_h = xpool.tile([128, 128], f16, name="fpt_h", tag="fpt_h")
                fpt_l = xpool.tile([64, 128], f16, name="fpt_l", tag="fpt_l")
                for (src, dsth, dstl) in ((ep_sb, ept_h, ept_l), (fp_sb, fpt_h, fpt_l)):
                    tp1 = ps.tile([128, 128], f16, name="tp1", tag="sb", bufs=4)
                    nc.tensor.transpose(
                        tp1[:], src[:, b * C : b * C + 128], ident[:]
                    )
                    nc.vector.tensor_copy(dsth[:], tp1[:])
                    tp2 = ps.tile([64, 128], f16, name="tp2", tag="sb", bufs=4)
                    nc.tensor.transpose(
                        tp2[:], src[:, b * C + 128 : b * C + 192], ident[:]
                    )
                    nc.vector.tensor_copy(dstl[:], tp2[:])

                # k_lowT = WkT.T @ EPT + const_kT  (feature-major (kch, r))
                kl_hi = ps.tile([128, R], f32, name="kl_hi", tag="sb", bufs=4)
                nc.tensor.matmul(kl_hi[:], wkt[:, 0:128], ept_h[:], start=True, stop=False)
                nc.tensor.matmul(kl_hi[:], wkt_l[:, 0:128], ept_l[:], start=False, stop=True)
                nc.vector.tensor_tensor(
                    klo_h[b][:], kl_hi[:], ckt_h[:], op=mybir.AluOpType.add
                )
                kl_lo = ps.tile([64, R], f32, name="kl_lo", tag="sb", bufs=4)
                nc.tensor.matmul(kl_lo[:], wkt[:, 128:192], ept_h[:], start=True, stop=False)
                nc.tensor.matmul(kl_lo[:], wkt_l[:, 128:192], ept_l[:], start=False, stop=True)
                nc.vector.tensor_tensor(
                    klo_l[b][:], kl_lo[:], ckt_l[:], op=mybir.AluOpType.add
                )
                # v_low (R-major (r, vch)), straight to f16 with const add
                vl_ps = ps.tile([128, C], f32, name="vl_ps", tag="sb", bufs=4)
                nc.tensor.matmul(vl_ps[:], fpt_h[:], wvt[:], start=True, stop=False)
                nc.tensor.matmul(vl_ps[:], fpt_l[:], wvt_l[:], start=False, stop=True)
                nc.vector.tensor_tensor(
                    vlo[b][:], vl_ps[:], cv[:], op=mybir.AluOpType.add
                )

            # ---------------- Phase B: attention tiles ----------
            def front1(b, t, qsb):
                """scores + exps for tile (b, t)"""
                W = 512 if t < 6 else 128
                base = b * NP + t * 512
                sps = []
                for h in range(6):
                    s1 = ps.tile([128, W], f32, name=f"s{h}", tag="sb", bufs=4)
                    if h < 4:
                        nc.tensor.matmul(
                            s1[:],
                            klo_h[b][32 * h : 32 * h + 32, :],
                            qsb[32 * h : 32 * h + 32, 0:W],
                            start=True, stop=True,
                            tile_position=(32 * h, 0),
                        )
                    else:
                        hh = h - 4
                        nc.tensor.matmul(
                            s1[:],
                            klo_l[b][32 * hh : 32 * hh + 32, :],
                            qsb[32 * hh : 32 * hh + 32, W : 2 * W],
                            start=True, stop=True,
                            tile_position=(32 * hh, 0),
                        )
                    sp1 = sppool.tile([128, W], f16, name=f"sp{h}", tag=f"sp{h}")
                    nc.scalar.activation(
                        sp1[:], s1[:], mybir.ActivationFunctionType.Exp
                    )
                    sps.append(sp1)
                return dict(W=W, base=base, b=b, t=t, sps=sps)

            def front2(st):
                """attn @ v_low + denominators for tile (b, t)"""
                W, b, t, sps = st["W"], st["b"], st["t"], st["sps"]
                avA = ps.tile([128, W], f32, name="avA", tag="bk", bufs=4)
                for h in range(4):
                    nc.tensor.matmul(
                        avA[32 * h : 32 * h + 32, :],
                        vlo[b][:, 32 * h : 32 * h + 32],
                        sps[h][:],
                        start=True, stop=True,
                        tile_position=(0, 32 * h),
                    )
                zA = ps.tile([128, W], f32, name="zA", tag="bk", bufs=4)
                for h in range(4):
                    nc.tensor.matmul(
                        zA[32 * h : 32 * h + 32, :],
                        ones_att[:],
                        sps[h][:],
                        start=True, stop=True,
                        tile_position=(0, 32 * h),
                    )
                av2 = ps.tile([64, W], f32, name="av2", tag="bk", bufs=4)
                for hh in range(2):
                    nc.tensor.matmul(
                        av2[32 * hh : 32 * hh + 32, :],
                        vlo[b][:, 128 + 32 * hh : 160 + 32 * hh],
                        sps[4 + hh][:],
                        start=True, stop=True,
                        tile_position=(0, 32 * hh),
                    )
                z2 = ps.tile([64, W], f32, name="z2", tag="bk", bufs=4)
                for hh in range(2):
                    nc.tensor.matmul(
                        z2[32 * hh : 32 * hh + 32, :],
                        ones_att[:],
                        sps[4 + hh][:],
                        start=True, stop=True,
                        tile_position=(0, 32 * hh),
                    )
                av_hi = avpool.tile([128, W], f16, name="av_hi", tag="av_hi")
                av_lo = av_lo_bufs[(b * 7 + t) % 2]
                st.update(avA=avA, zA=zA, av2=av2, z2=z2, av_hi=av_hi, av_lo=av_lo)

            osb_state = {}

            def back(st):
                W, base, t = st["W"], st["base"], st["t"]
                rzA = divpool.tile([128, W], f32, name="rzA", tag="rzA")
                nc.vector.reciprocal_approx_fast(rzA[:], st["zA"][:])
                rz2 = divpool.tile([64, W], f32, name="rz2", tag="rz2")
                nc.vector.reciprocal_approx_fast(rz2[:], st["z2"][:])
                av_hi, av_lo = st["av_hi"], st["av_lo"]
                nc.vector.tensor_tensor(
                    av_hi[:, :], st["avA"][:], rzA[:], op=mybir.AluOpType.mult
                )
                nc.vector.tensor_tensor(
                    av_lo[0:64, 0:W], st["av2"][:], rz2[:], op=mybir.AluOpType.mult
                )
                # output projection: proj weights stationary, av moving; the
                # psum outputs alias the (already consumed) avA / zA banks.
                o1 = st["avA"]
                nc.tensor.matmul(o1[:], pw_hh[:], av_hi[:, 0:W], start=True, stop=False)
                nc.tensor.matmul(o1[:], pw_lh[:], av_lo[:, 0:W], start=False, stop=True)
                o2 = st["zA"][0:64, :]
                nc.tensor.matmul(o2, pw_hl[:], av_hi[:, 0:W], start=True, stop=False)
                nc.tensor.matmul(o2, pw_ll[:], av_lo[:, 0:W], start=False, stop=True)
                # stage pairs of tiles and DMA once per pair; the hi copy runs
                # on ACT, the lo copy on DVE (engine balance)
                if t % 2 == 0:
                    osb_state["hi"] = opool.tile([128, 1024], f16, name="osb", tag="osb")
                    osb_state["lo"] = opool.tile([64, 1024], f16, name="osb2", tag="osb2")
                    osb_state["base"] = base
                off = (t % 2) * 512
                osb, osb2 = osb_state["hi"], osb_state["lo"]
                nc.scalar.copy(osb[:, off : off + W], o1[:])
                nc.vector.tensor_copy(osb2[:, off : off + W], o2)
                if t % 2 == 1 or t == 6:
                    w_tot = 512 + W if t % 2 == 1 else W
                    b0 = osb_state["base"]
                    nc.gpsimd.dma_start(
                        outh_d[:, b0 : b0 + w_tot], osb[:, 0:w_tot]
                    )
                    nc.gpsimd.dma_start(
                        outl_d[:, b0 : b0 + w_tot], osb2[:, 0:w_tot]
                    )

            tiles = [(b, t) for b in range(B_PER) for t in range(7)]
            prev = None
            qsb_cur = qproj(0, 0)
            for i, (b, t) in enumerate(tiles):
                st = front1(b, t, qsb_cur)
                if prev is not None:
                    back(prev)
                if i + 1 < len(tiles):
                    bn, tn = tiles[i + 1]
                    qsb_cur = qproj(bn, tn)
                front2(st)
                prev = st
            back(prev)

    nc.compile()
    return nc


def _get_nc():
    if "nc" not in _STATE:
        _STATE["nc"] = _build_bass()
    return _STATE["nc"]


def kernel(x, qkv_w, qkv_b, E_w, E_b, F_w, F_b, proj_w, proj_b, h, w):
    from concourse.bass_utils import run_bass_kernel_spmd

    x = np.asarray(x, dtype=np.float32)
    qkv_w = np.asarray(qkv_w, dtype=np.float32)
    qkv_b = np.asarray(qkv_b, dtype=np.float32)
    E_w = np.asarray(E_w, dtype=np.float32)
    E_b = np.asarray(E_b, dtype=np.float32)
    F_w = np.asarray(F_w, dtype=np.float32)
    F_b = np.asarray(F_b, dtype=np.float32)
    proj_w = np.asarray(proj_w, dtype=np.float32)
    proj_b = np.asarray(proj_b, dtype=np.float32)
    assert int(h) == 56 and int(w) == 56

    n_of_m = _window_perm()
    E_wx = np.ascontiguousarray(E_w[:, n_of_m])
    F_wx = np.ascontiguousarray(F_w[:, n_of_m])

    Wq, Wk, Wv = qkv_w[0:C], qkv_w[C : 2 * C], qkv_w[2 * C : 3 * C]
    bq, bk, bv = qkv_b[0:C], qkv_b[C : 2 * C], qkv_b[2 * C : 3 * C]
    scale = np.float32(1.0 / np.sqrt(HD))

    const_k = np.outer(E_wx.sum(1), bk) + E_b[:, None]      # (128, 192)
    const_v = (np.outer(F_wx.sum(1), bv) + F_b[:, None]).astype(np.float32)

    import ml_dtypes

    f8np = ml_dtypes.float8_e4m3
    # q weights in fp8 DoubleRow layout [97, 2, m] with a 2^7 gain; row 96
    # carries the (scaled, gained) bias, paired with a ones-row in xq8.
    wqs_t = np.ascontiguousarray((Wq * scale).T * 128.0)      # (c, m)
    wq_dr = np.zeros((97, 2, C), dtype=np.float32)
    wq_dr[0:96] = wqs_t.reshape(2, 96, C).transpose(1, 0, 2)
    wq_dr[96, 0] = bq * scale * 128.0
    wq8_hi = np.ascontiguousarray(wq_dr[:, :, 0:128]).reshape(97, 256).astype(f8np)
    wq8_lo = np.ascontiguousarray(wq_dr[:, :, 128:192]).reshape(97, 128).astype(f8np)
    # 2^-7 gain undo baked into the k_low path
    wkt = np.ascontiguousarray(Wk.T * (2.0 ** -7)).astype(np.float16)
    wvt = np.ascontiguousarray(Wv.T).astype(np.float16)
    ckt = np.ascontiguousarray(const_k.T.astype(np.float32) * (2.0 ** -7))  # (192, 128)
    pw = proj_w.T                                            # (ch, co)
    pw_hh = np.ascontiguousarray(pw[0:128, 0:128]).astype(np.float16)
    pw_hl = np.ascontiguousarray(pw[0:128, 128:192]).astype(np.float16)
    pw_lh = np.zeros((65, 128), dtype=np.float16)
    pw_lh[0:64] = pw[128:192, 0:128]
    pw_lh[64] = proj_b[0:128]
    pw_ll = np.zeros((65, 64), dtype=np.float16)
    pw_ll[0:64] = pw[128:192, 128:192]
    pw_ll[64] = proj_b[128:192]

    e_wxt_full = np.ascontiguousarray(E_wx.T).astype(np.float16)  # (3136, 128)
    f_wxt_full = np.ascontiguousarray(F_wx.T).astype(np.float16)
    # pre-chunked layout: (24, 128, R) -> (128, 24*R) so each SBUF partition
    # line is one contiguous DMA descriptor
    e_wxt = np.ascontiguousarray(
        e_wxt_full[0:3072].reshape(24, 128, R).transpose(1, 0, 2).reshape(128, 24 * R)
    )
    f_wxt = np.ascontiguousarray(
        f_wxt_full[0:3072].reshape(24, 128, R).transpose(1, 0, 2).reshape(128, 24 * R)
    )
    e_tl = np.ascontiguousarray(e_wxt_full[3072:3136])
    f_tl = np.ascontiguousarray(f_wxt_full[3072:3136])
    ident = np.eye(128, dtype=np.float16)
    ones_att = np.ones((128, 32), dtype=np.float16)
    ones_row = np.ones((1, 512), dtype=np.float16)

    consts = dict(
        e_wxt=e_wxt, f_wxt=f_wxt, e_tl=e_tl, f_tl=f_tl,
        wq8_hi=wq8_hi, wq8_lo=wq8_lo, wkt=wkt, wvt=wvt,
        const_kt=ckt, const_v=const_v,
        pw_hh=pw_hh, pw_hl=pw_hl, pw_lh_aug=pw_lh, pw_ll_aug=pw_ll,
        ident=ident, ones_att=ones_att, ones_row=ones_row,
    )

    # shard x: core i gets batches 4i..4i+4, padded to NP tokens per batch
    xb = x.reshape(B_TOT, 64 * 49, C).astype(np.float16)
    in_maps = []
    for i in range(N_CORES):
        xi = np.zeros((B_PER, NP, C), dtype=np.float16)
        xi[:, 0:N, :] = xb[B_PER * i : B_PER * (i + 1)]
        # phase-A layout: row n = 4 batches' channels side by side
        xa = np.ascontiguousarray(xi.transpose(1, 0, 2).reshape(NP, 4 * C))
        M = B_PER * NP
        xt = xi.reshape(M, C).T                               # (c, M)
        xq8 = np.zeros((97, 2, M), dtype=np.float32)
        xq8[0:96] = xt.reshape(2, 96, M).transpose(1, 0, 2)
        xq8[96, 0] = 1.0
        xq8 = np.ascontiguousarray(xq8.reshape(97, 2 * M)).astype(f8np)
        in_maps.append({**consts, "x_a": xa, "xq8": xq8})

    nc = _get_nc()
    _STATE["last_in_maps"] = in_maps
    res = run_bass_kernel_spmd(nc, in_maps, core_ids=list(range(N_CORES)))

    out_win = np.empty((B_TOT, N, C), dtype=np.float32)
    for i in range(N_CORES):
        oh = res.results[i]["out_hi"].astype(np.float32)      # (128, B_PER*NP)
        ol = res.results[i]["out_lo"].astype(np.float32)      # (64, B_PER*NP)
        oc = np.concatenate([oh, ol], axis=0)                 # (192, B_PER*NP)
        oi = oc.reshape(C, B_PER, NP).transpose(1, 2, 0)      # (B_PER, NP, C)
        out_win[B_PER * i : B_PER * (i + 1)] = oi[:, 0:N, :]
    # window_reverse on the gathered output
    out_sp = (
        out_win.reshape(B_TOT, 8, 8, 7, 7, C)
        .transpose(0, 1, 3, 2, 4, 5)
        .reshape(B_TOT, N, C)
    )
    return np.ascontiguousarray(out_sp)


# revision 17
# speedup vs baseline: 1.2667x; 1.0888x over previous
"""Trainium2 Bass kernel for nn_CompleteAttention_68418829025814.

Linformer-style windowed attention, restructured for the PE array:
  - window_reverse is folded into a host-side column permutation of E_w/F_w
    (device works entirely in x's native window order) and a host-side
    permutation of the gathered output.
  - k/v are never materialized: k_low = (E @ x) @ Wk^T + const (the E/F
    projections contract over tokens, so x is used in its native layout).
  - q is never materialized either: scores_h = (k_low_h^T Wq_h scale) @ x^T
    = M_h @ x^T, with the tiny per-batch M_h = [c, r] built on device and
    the q bias folded into the exp's per-partition bias vector.
  - phase A contracts all 4 batches at once ([128, 768] x-chunks, double
    chunks per DMA on the otherwise-idle scalar queue), so k_low/v_low/M
    for every batch are ready ~25us in and phase B covers the rest.
  - phase B per-tile emission is ordered for the in-order engine queues:
    scores+exps(t) -> back(t-1) (recip/divide/proj) -> av/z(t), keeping the
    PE fed (proj of t-1) while ACT runs t's exps.
  - output projection runs with proj_w stationary; outputs land feature-major
    [c_out, token] (psum banks aliased onto the consumed avA/zA tiles) and
    the final transpose happens on host.
  - HW constraints found the hard way: PSUM tensors must be single-bank, and
    engine reads of PSUM must start at partition 0.

Sharding: data-parallel over batch; each of the 8 cores gets 4 batches
(256 windows) of x. Small weights are replicated.
"""

import numpy as np

B_TOT = 32
N_CORES = 8
B_PER = B_TOT // N_CORES      # 4 batches per core
N = 3136                      # tokens per batch
NP = 3200                     # padded tokens per batch (6*512 + 128)
C = 192
H = 6
HD = 32
R = 128
WS = 7

_STATE = {}


def _window_perm():
    """n_of_m[m] = spatial index n for window-order position m."""
    hh, ww, i, j = np.meshgrid(
        np.arange(8), np.arange(8), np.arange(7), np.arange(7), indexing="ij"
    )
    m = (hh * 8 + ww) * 49 + i * 7 + j
    n = (hh * 7 + i) * 56 + ww * 7 + j
    n_of_m = np.empty(N, dtype=np.int64)
    n_of_m[m.ravel()] = n.ravel()
    return n_of_m


def _build_bass():
    import concourse.bacc as bacc
    import concourse.mybir as mybir
    from concourse.tile import TileContext

    f32 = mybir.dt.float32
    f16 = mybir.dt.float16

    nc = bacc.Bacc("TRN2", target_bir_lowering=False, debug=False)

    # x_a: phase-A layout, row n = [x[0, n, :] | x[1, n, :] | x[2, n, :] | x[3, n, :]]
    x_d = nc.dram_tensor("x_a", [NP, 4 * C], f16, kind="ExternalInput")
    # x^T for the scores: rows = channels (hi 0:128; lo = 128:192 duplicated
    # onto partitions 0-63 and 64-127 for row-band pairing)
    xth_d = nc.dram_tensor("xt_hi", [128, B_PER * NP], f16, kind="ExternalInput")
    xtl_d = nc.dram_tensor("xt_lo2", [128, B_PER * NP], f16, kind="ExternalInput")
    # e/f shipped pre-chunked: row p = 24 chunks of 128 R-values (token 128k+p)
    e_d = nc.dram_tensor("e_wxt", [128, 24 * R], f16, kind="ExternalInput")
    f_d = nc.dram_tensor("f_wxt", [128, 24 * R], f16, kind="ExternalInput")
    e_tl_d = nc.dram_tensor("e_tl", [64, R], f16, kind="ExternalInput")
    f_tl_d = nc.dram_tensor("f_tl", [64, R], f16, kind="ExternalInput")
    wkt_d = nc.dram_tensor("wkt", [C, C], f16, kind="ExternalInput")
    wvt_d = nc.dram_tensor("wvt", [C, C], f16, kind="ExternalInput")
    ckt_d = nc.dram_tensor("const_kt", [C, R], f32, kind="ExternalInput")
    cv_d = nc.dram_tensor("const_v", [R, C], f32, kind="ExternalInput")
    # Wq (scaled) laid out by q-channel rows; used to build M_h on device
    wqh_d = nc.dram_tensor("wq_hi_dc", [128, C], f16, kind="ExternalInput")
    wql_d = nc.dram_tensor("wq_lo_dc", [64, C], f16, kind="ExternalInput")
    # block-diagonal q-bias columns for the exp bias build
    bqh_d = nc.dram_tensor("bqblk_hi", [128, 4], f16, kind="ExternalInput")
    bql_d = nc.dram_tensor("bqblk_lo", [64, 2], f16, kind="ExternalInput")
    # proj weights, stationary chunks: pw = proj_w.T (ch, co)
    pwhh_d = nc.dram_tensor("pw_hh", [128, 128], f16, kind="ExternalInput")
    pwhl_d = nc.dram_tensor("pw_hl", [128, 64], f16, kind="ExternalInput")
    pwlh_d = nc.dram_tensor("pw_lh_aug", [65, 128], f16, kind="ExternalInput")
    pwll_d = nc.dram_tensor("pw_ll_aug", [65, 64], f16, kind="ExternalInput")
    ident_d = nc.dram_tensor("ident", [128, 128], f16, kind="ExternalInput")
    ones_d = nc.dram_tensor("ones_att", [128, 32], f16, kind="ExternalInput")
    onesrow_d = nc.dram_tensor("ones_row", [1, 512], f16, kind="ExternalInput")
    # outputs feature-major: out[c, b*NP + n]
    outh_d = nc.dram_tensor("out_hi", [128, B_PER * NP], f16, kind="ExternalOutput")
    outl_d = nc.dram_tensor("out_lo", [64, B_PER * NP], f16, kind="ExternalOutput")

    NCH = 25  # n-chunks per batch for the E/F contraction (24*128 + 64)

    with TileContext(nc) as tc:
        with tc.tile_pool(name="const", bufs=1) as cpool, \
             tc.tile_pool(name="ef", bufs=1) as efpool, \
             tc.tile_pool(name="low", bufs=1) as lowpool, \
             tc.tile_pool(name="mh", bufs=1) as mhpool, \
             tc.tile_pool(name="xin", bufs=4) as xpool, \
             tc.tile_pool(name="xt", bufs=3) as xtpool, \
             tc.tile_pool(name="sp", bufs=2) as sppool, \
             tc.tile_pool(name="div", bufs=2) as divpool, \
             tc.tile_pool(name="av", bufs=2) as avpool, \
             tc.tile_pool(name="osb", bufs=2) as opool, \
             tc.tile_pool(name="ps", bufs=4, space="PSUM") as ps:

            # ---- constants (sync queue; scalar queue is reserved for the
            # phase-A x stream, gpsimd for e/f) ----
            ident = cpool.tile([128, 128], f16)
            nc.sync.dma_start(ident[:], ident_d[:])
            wkt = cpool.tile([128, C], f16)
            nc.sync.dma_start(wkt[:], wkt_d[0:128, :])
            wkt_l = cpool.tile([64, C], f16)
            nc.sync.dma_start(wkt_l[:], wkt_d[128:192, :])
            wvt = cpool.tile([128, C], f16)
            nc.sync.dma_start(wvt[:], wvt_d[0:128, :])
            wvt_l = cpool.tile([64, C], f16)
            nc.sync.dma_start(wvt_l[:], wvt_d[128:192, :])
            ckt_h = cpool.tile([128, R], f32)
            nc.sync.dma_start(ckt_h[:], ckt_d[0:128, :])
            ckt_l = cpool.tile([64, R], f32)
            nc.sync.dma_start(ckt_l[:], ckt_d[128:192, :])
            cv = cpool.tile([128, C], f32)
            nc.sync.dma_start(cv[:], cv_d[:])
            wq_hi = cpool.tile([128, C], f16)
            nc.sync.dma_start(wq_hi[:], wqh_d[:])
            wq_lo = cpool.tile([64, C], f16)
            nc.sync.dma_start(wq_lo[:], wql_d[:])
            bq_hi = cpool.tile([128, 4], f16)
            nc.sync.dma_start(bq_hi[:], bqh_d[:])
            bq_lo = cpool.tile([64, 2], f16)
            nc.sync.dma_start(bq_lo[:], bql_d[:])
            pw_hh = cpool.tile([128, 128], f16)
            nc.sync.dma_start(pw_hh[:], pwhh_d[:])
            pw_hl = cpool.tile([128, 64], f16)
            nc.sync.dma_start(pw_hl[:], pwhl_d[:])
            pw_lh = cpool.tile([65, 128], f16)
            nc.sync.dma_start(pw_lh[:], pwlh_d[:])
            pw_ll = cpool.tile([65, 64], f16)
            nc.sync.dma_start(pw_ll[:], pwll_d[:])
            ones_att = cpool.tile([128, 32], f16)
            nc.sync.dma_start(ones_att[:], ones_d[:])
            # persistent [65, 512] attn-output staging tiles whose row 64
            # stays 1.0 forever (feeds proj_b through pw_l*_aug's last row)
            av_lo_bufs = [cpool.tile([65, 512], f16, name=f"avlo{i}") for i in range(2)]
            for i in range(2):
                nc.sync.dma_start(av_lo_bufs[i][64:65, :], onesrow_d[:])

            # E/F transposed weights resident in SBUF: 24 full chunks + tail
            e_sb = efpool.tile([128, 24, 128], f16)
            f_sb = efpool.tile([128, 24, 128], f16)
            e_tl = efpool.tile([64, 128], f16)
            f_tl = efpool.tile([64, 128], f16)

            def load_ef_group(g):
                sl = slice(g * 6, (g + 1) * 6)
                dsl = slice(g * 6 * 128, (g + 1) * 6 * 128)
                nc.gpsimd.dma_start(
                    e_sb[:, sl, :].rearrange("p k r -> p (k r)"), e_d[:, dsl]
                )
                nc.gpsimd.dma_start(
                    f_sb[:, sl, :].rearrange("p k r -> p (k r)"), f_d[:, dsl]
                )

            for g in range(4):
                load_ef_group(g)
            nc.gpsimd.dma_start(e_tl[:], e_tl_d[:])
            nc.gpsimd.dma_start(f_tl[:], f_tl_d[:])

            # per-batch low-rank tensors (kept resident across phase B)
            klo_h = [lowpool.tile([128, R], f16, name=f"klo_h{b}") for b in range(B_PER)]
            klo_l = [lowpool.tile([64, R], f16, name=f"klo_l{b}") for b in range(B_PER)]
            vlo = [lowpool.tile([128, C], f16, name=f"vlo{b}") for b in range(B_PER)]

            # ---------------- Phase A: EP/FP for all 4 batches ----------------
            # x chunks are [nk, 768] = 4 batches wide; two chunks per DMA on
            # the dedicated scalar queue.
            ep_a = ps.tile([128, 2 * C], f32, name="ep_a", tag="bk", bufs=4)
            ep_b = ps.tile([128, 2 * C], f32, name="ep_b", tag="bk", bufs=4)
            fp_a = ps.tile([128, 2 * C], f32, name="fp_a", tag="bk", bufs=4)
            fp_b = ps.tile([128, 2 * C], f32, name="fp_b", tag="bk", bufs=4)
            for cp in range(13):
                k = 2 if cp < 12 else 1
                x2 = xpool.tile([128, 2, 4 * C], f16, name="x2", tag="x2")
                nc.scalar.dma_start(
                    x2[:, 0:k, :],
                    x_d[cp * 256 : cp * 256 + k * 128, :].rearrange(
                        "(k p) c -> p k c", p=128
                    ),
                )
                for kk in range(k):
                    ci = cp * 2 + kk
                    nk = 128 if ci < 24 else 64
                    elh = e_sb[:, ci, :] if ci < 24 else e_tl[:]
                    flh = f_sb[:, ci, :] if ci < 24 else f_tl[:]
                    x2f = x2[0:nk, kk, :]
                    st = ci == 0
                    sp_ = ci == NCH - 1
                    nc.tensor.matmul(
                        ep_a[:], elh, x2f[:, 0 : 2 * C], start=st, stop=sp_
                    )
                    nc.tensor.matmul(
                        fp_a[:], flh, x2f[:, 0 : 2 * C], start=st, stop=sp_
                    )
                    nc.tensor.matmul(
                        ep_b[:], elh, x2f[:, 2 * C : 4 * C], start=st, stop=sp_
                    )
                    nc.tensor.matmul(
                        fp_b[:], flh, x2f[:, 2 * C : 4 * C], start=st, stop=sp_
                    )
            ep_sb = xpool.tile([128, 4 * C], f16, name="ep_sb", tag="ep_sb")
            nc.vector.tensor_copy(ep_sb[:, 0 : 2 * C], ep_a[:])
            nc.vector.tensor_copy(ep_sb[:, 2 * C : 4 * C], ep_b[:])
            fp_sb = xpool.tile([128, 4 * C], f16, name="fp_sb", tag="fp_sb")
            nc.vector.tensor_copy(fp_sb[:, 0 : 2 * C], fp_a[:])
            nc.vector.tensor_copy(fp_sb[:, 2 * C : 4 * C], fp_b[:])

            # per-batch M_h = [c, r] score matrices + exp bias vectors
            mh_hi = [mhpool.tile([128, 6, 128], f16, name=f"mh_hi{b}")
                     for b in range(B_PER)]
            mh_lo = [mhpool.tile([128, 6, 128], f16, name=f"mh_lo{b}")
                     for b in range(B_PER)]
            cb_sb = [mhpool.tile([128, 8], f32, name=f"cb{b}")
                     for b in range(B_PER)]

            for b in range(B_PER):
                # transpose EP, FP slices: (r=128, c=192) -> (c, r)
                ept_h = xpool.tile([128, 128], f16, name="ept_h", tag="ept_h")
                ept_l = xpool.tile([64, 128], f16, name="ept_l", tag="ept_l")
                fpt_h = xpool.tile([128, 128], f16, name="fpt_h", tag="fpt_h")
                fpt_l = xpool.tile([64, 128], f16, name="fpt_l", tag="fpt_l")
                for (src, dsth, dstl) in ((ep_sb, ept_h, ept_l), (fp_sb, fpt_h, fpt_l)):
                    tp1 = ps.tile([128, 128], f16, name="tp1", tag="sb", bufs=4)
                    nc.tensor.transpose(
                        tp1[:], src[:, b * C : b * C + 128], ident[:]
                    )
                    nc.vector.tensor_copy(dsth[:], tp1[:])
                    tp2 = ps.tile([64, 128], f16, name="tp2", tag="sb", bufs=4)
                    nc.tensor.transpose(
                        tp2[:], src[:, b * C + 128 : b * C + 192], ident[:]
                    )
                    nc.vector.tensor_copy(dstl[:], tp2[:])

                # k_lowT = WkT.T @ EPT + const_kT  (feature-major (kch, r))
                kl_hi = ps.tile([128, R], f32, name="kl_hi", tag="sb", bufs=4)
                nc.tensor.matmul(kl_hi[:], wkt[:, 0:128], ept_h[:], start=True, stop=False)
                nc.tensor.matmul(kl_hi[:], wkt_l[:, 0:128], ept_l[:], start=False, stop=True)
                nc.vector.tensor_tensor(
                    klo_h[b][:], kl_hi[:], ckt_h[:], op=mybir.AluOpType.add
                )
                kl_lo = ps.tile([64, R], f32, name="kl_lo", tag="sb", bufs=4)
                nc.tensor.matmul(kl_lo[:], wkt[:, 128:192], ept_h[:], start=True, stop=False)
                nc.tensor.matmul(kl_lo[:], wkt_l[:, 128:192], ept_l[:], start=False, stop=True)
                nc.vector.tensor_tensor(
                    klo_l[b][:], kl_lo[:], ckt_l[:], op=mybir.AluOpType.add
                )
                # v_low (R-major (r, vch)), straight to f16 with const add
                vl_ps = ps.tile([128, C], f32, name="vl_ps", tag="sb", bufs=4)
                nc.tensor.matmul(vl_ps[:], fpt_h[:], wvt[:], start=True, stop=False)
                nc.tensor.matmul(vl_ps[:], fpt_l[:], wvt_l[:], start=False, stop=True)
                nc.vector.tensor_tensor(
                    vlo[b][:], vl_ps[:], cv[:], op=mybir.AluOpType.add
                )

                # M_h = klo_h^T @ Wq_h (both indexed by q-channel rows), laid
                # out [c, r]; the c 128:192 part is duplicated onto partitions
                # 64-127 so the lo score matmuls can run band-paired.
                for h in range(6):
                    if h < 4:
                        kslice = klo_h[b][32 * h : 32 * h + 32, :]
                        wslice_hi = wq_hi[32 * h : 32 * h + 32, 0:128]
                        wslice_lo = wq_hi[32 * h : 32 * h + 32, 128:192]
                        tp = (32 * h, 0)
                    else:
                        hh = h - 4
                        kslice = klo_l[b][32 * hh : 32 * hh + 32, :]
                        wslice_hi = wq_lo[32 * hh : 32 * hh + 32, 0:128]
                        wslice_lo = wq_lo[32 * hh : 32 * hh + 32, 128:192]
                        tp = (32 * hh, 0)
                    m1 = ps.tile([128, 128], f32, name="m1", tag="sb", bufs=4)
                    nc.tensor.matmul(m1[:], wslice_hi, kslice, start=True,
                                     stop=True, tile_position=tp)
                    nc.vector.tensor_copy(mh_hi[b][:, h, :], m1[:])
                    m2 = ps.tile([128, 128], f32, name="m2", tag="sb", bufs=4)
                    nc.tensor.matmul(m2[0:64, :], wslice_lo, kslice, start=True,
                                     stop=True, tile_position=(tp[0], 0))
                    nc.tensor.matmul(m2[64:128, :], wslice_lo, kslice, start=True,
                                     stop=True, tile_position=(tp[0], 64))
                    nc.vector.tensor_copy(mh_lo[b][:, h, :], m2[:])
                # exp bias: cb[r, h] = sum_d klo[d, r] * bq_scaled[d]
                cbp = ps.tile([128, 8], f32, name="cbp", tag="sb", bufs=4)
                nc.tensor.matmul(cbp[:, 0:4], klo_h[b][:], bq_hi[:],
                                 start=True, stop=True)
                nc.tensor.matmul(cbp[:, 4:6], klo_l[b][:], bq_lo[:],
                                 start=True, stop=True)
                nc.vector.tensor_copy(cb_sb[b][:, 0:6], cbp[:, 0:6])

            # ---------------- Phase B: attention tiles ----------
            def front1(b, t):
                """x^T loads + scores + exps for tile (b, t)"""
                W = 512 if t < 6 else 128
                base = b * NP + t * 512
                xth = xtpool.tile([128, W], f16, name="xth", tag="xth")
                nc.sync.dma_start(xth[:], xth_d[:, base : base + W])
                xtl = xtpool.tile([128, W], f16, name="xtl", tag="xtl")
                nc.sync.dma_start(xtl[:], xtl_d[:, base : base + W])
                sps = []
                sbanks = []
                for h in range(6):
                    s1 = ps.tile([128, W], f32, name=f"s{h}", tag="sb", bufs=4)
                    nc.tensor.matmul(
                        s1[:], mh_hi[b][:, h, :], xth[:],
                        start=True, stop=False,
                    )
                    sbanks.append(s1)
                    if h % 2 == 1:
                        for j, hp in enumerate((h - 1, h)):
                            nc.tensor.matmul(
                                sbanks[hp][:],
                                mh_lo[b][64 * j : 64 * j + 64, hp, :],
                                xtl[64 * j : 64 * j + 64, :],
                                start=False, stop=True,
                                tile_position=(64 * j, 0),
                            )
                        for hp in (h - 1, h):
                            sp1 = sppool.tile(
                                [128, W], f16, name=f"sp{hp}", tag=f"sp{hp}"
                            )
                            nc.scalar.activation(
                                sp1[:], sbanks[hp][:],
                                mybir.ActivationFunctionType.Exp,
                                bias=cb_sb[b][:, hp : hp + 1],
                            )
                            sps.append(sp1)
                return dict(W=W, base=base, b=b, t=t, sps=sps)

            def front2(st):
                """attn @ v_low + denominators for tile (b, t)"""
                W, b, t, sps = st["W"], st["b"], st["t"], st["sps"]
                avA = ps.tile([128, W], f32, name="avA", tag="bk", bufs=4)
                for h in range(4):
                    nc.tensor.matmul(
                        avA[32 * h : 32 * h + 32, :],
                        vlo[b][:, 32 * h : 32 * h + 32],
                        sps[h][:],
                        start=True, stop=True,
                        tile_position=(0, 32 * h),
                    )
                zA = ps.tile([128, W], f32, name="zA", tag="bk", bufs=4)
                for h in range(4):
                    nc.tensor.matmul(
                        zA[32 * h : 32 * h + 32, :],
                        ones_att[:],
                        sps[h][:],
                        start=True, stop=True,
                        tile_position=(0, 32 * h),
                    )
                av2 = ps.tile([64, W], f32, name="av2", tag="bk", bufs=4)
                for hh in range(2):
                    nc.tensor.matmul(
                        av2[32 * hh : 32 * hh + 32, :],
                        vlo[b][:, 128 + 32 * hh : 160 + 32 * hh],
                        sps[4 + hh][:],
                        start=True, stop=True,
                        tile_position=(0, 32 * hh),
                    )
                z2 = ps.tile([64, W], f32, name="z2", tag="bk", bufs=4)
                for hh in range(2):
                    nc.tensor.matmul(
                        z2[32 * hh : 32 * hh + 32, :],
                        ones_att[:],
                        sps[4 + hh][:],
                        start=True, stop=True,
                        tile_position=(0, 32 * hh),
                    )
                av_hi = avpool.tile([128, W], f16, name="av_hi", tag="av_hi")
                av_lo = av_lo_bufs[(b * 7 + t) % 2]
                st.update(avA=avA, zA=zA, av2=av2, z2=z2, av_hi=av_hi, av_lo=av_lo)

            osb_state = {}

            def back(st):
                W, base, t = st["W"], st["base"], st["t"]
                rzA = divpool.tile([128, W], f32, name="rzA", tag="rzA")
                nc.vector.reciprocal_approx_fast(rzA[:], st["zA"][:])
                rz2 = divpool.tile([64, W], f32, name="rz2", tag="rz2")
                nc.vector.reciprocal_approx_fast(rz2[:], st["z2"][:])
                av_hi, av_lo = st["av_hi"], st["av_lo"]
                nc.vector.tensor_tensor(
                    av_hi[:, :], st["avA"][:], rzA[:], op=mybir.AluOpType.mult
                )
                nc.vector.tensor_tensor(
                    av_lo[0:64, 0:W], st["av2"][:], rz2[:], op=mybir.AluOpType.mult
                )
                # output projection: proj weights stationary, av moving; the
                # psum outputs alias the (already consumed) avA / zA banks.
                o1 = st["avA"]
                nc.tensor.matmul(o1[:], pw_hh[:], av_hi[:, 0:W], start=True, stop=False)
                nc.tensor.matmul(o1[:], pw_lh[:], av_lo[:, 0:W], start=False, stop=True)
                o2 = st["zA"][0:64, :]
                nc.tensor.matmul(o2, pw_hl[:], av_hi[:, 0:W], start=True, stop=False)
                nc.tensor.matmul(o2, pw_ll[:], av_lo[:, 0:W], start=False, stop=True)
                # stage pairs of tiles and DMA once per pair; the hi copy runs
                # on ACT, the lo copy on DVE (engine balance)
                if t % 2 == 0:
                    osb_state["hi"] = opool.tile([128, 1024], f16, name="osb", tag="osb")
                    osb_state["lo"] = opool.tile([64, 1024], f16, name="osb2", tag="osb2")
                    osb_state["base"] = base
                off = (t % 2) * 512
                osb, osb2 = osb_state["hi"], osb_state["lo"]
                nc.scalar.copy(osb[:, off : off + W], o1[:])
                nc.vector.tensor_copy(osb2[:, off : off + W], o2)
                if t % 2 == 1 or t == 6:
                    w_tot = 512 + W if t % 2 == 1 else W
                    b0 = osb_state["base"]
                    nc.gpsimd.dma_start(
                        outh_d[:, b0 : b0 + w_tot], osb[:, 0:w_tot]
                    )
                    nc.gpsimd.dma_start(
                        outl_d[:, b0 : b0 + w_tot], osb2[:, 0:w_tot]
                    )

            tiles = [(b, t) for b in range(B_PER) for t in range(7)]
            prev = None
            for b, t in tiles:
                st = front1(b, t)
                if prev is not None:
                    back(prev)
                front2(st)
                prev = st
            back(prev)

    nc.compile()
    return nc


def _get_nc():
    if "nc" not in _STATE:
        _STATE["nc"] = _build_bass()
    return _STATE["nc"]


def kernel(x, qkv_w, qkv_b, E_w, E_b, F_w, F_b, proj_w, proj_b, h, w):
    from concourse.bass_utils import run_bass_kernel_spmd

    x = np.asarray(x, dtype=np.float32)
    qkv_w = np.asarray(qkv_w, dtype=np.float32)
    qkv_b = np.asarray(qkv_b, dtype=np.float32)
    E_w = np.asarray(E_w, dtype=np.float32)
    E_b = np.asarray(E_b, dtype=np.float32)
    F_w = np.asarray(F_w, dtype=np.float32)
    F_b = np.asarray(F_b, dtype=np.float32)
    proj_w = np.asarray(proj_w, dtype=np.float32)
    proj_b = np.asarray(proj_b, dtype=np.float32)
    assert int(h) == 56 and int(w) == 56

    n_of_m = _window_perm()
    E_wx = np.ascontiguousarray(E_w[:, n_of_m])
    F_wx = np.ascontiguousarray(F_w[:, n_of_m])

    Wq, Wk, Wv = qkv_w[0:C], qkv_w[C : 2 * C], qkv_w[2 * C : 3 * C]
    bq, bk, bv = qkv_b[0:C], qkv_b[C : 2 * C], qkv_b[2 * C : 3 * C]
    scale = np.float32(1.0 / np.sqrt(HD))

    const_k = np.outer(E_wx.sum(1), bk) + E_b[:, None]      # (128, 192)
    const_v = (np.outer(F_wx.sum(1), bv) + F_b[:, None]).astype(np.float32)

    # Wq rows are q-channels; scaled by 1/sqrt(hd). M_h is built on device.
    wq_hi_dc = np.ascontiguousarray(Wq[0:128, :] * scale).astype(np.float16)
    wq_lo_dc = np.ascontiguousarray(Wq[128:192, :] * scale).astype(np.float16)
    bqblk_hi = np.zeros((128, 4), dtype=np.float16)
    for hh in range(4):
        bqblk_hi[32 * hh : 32 * hh + 32, hh] = (bq[32 * hh : 32 * hh + 32] * scale)
    bqblk_lo = np.zeros((64, 2), dtype=np.float16)
    for hh in range(2):
        bqblk_lo[32 * hh : 32 * hh + 32, hh] = (
            bq[128 + 32 * hh : 160 + 32 * hh] * scale
        )

    wkt = np.ascontiguousarray(Wk.T).astype(np.float16)
    wvt = np.ascontiguousarray(Wv.T).astype(np.float16)
    ckt = np.ascontiguousarray(const_k.T.astype(np.float32))  # (192, 128)
    pw = proj_w.T                                            # (ch, co)
    pw_hh = np.ascontiguousarray(pw[0:128, 0:128]).astype(np.float16)
    pw_hl = np.ascontiguousarray(pw[0:128, 128:192]).astype(np.float16)
    pw_lh = np.zeros((65, 128), dtype=np.float16)
    pw_lh[0:64] = pw[128:192, 0:128]
    pw_lh[64] = proj_b[0:128]
    pw_ll = np.zeros((65, 64), dtype=np.float16)
    pw_ll[0:64] = pw[128:192, 128:192]
    pw_ll[64] = proj_b[128:192]

    e_wxt_full = np.ascontiguousarray(E_wx.T).astype(np.float16)  # (3136, 128)
    f_wxt_full = np.ascontiguousarray(F_wx.T).astype(np.float16)
    # pre-chunked layout: (24, 128, R) -> (128, 24*R) so each SBUF partition
    # line is one contiguous DMA descriptor
    e_wxt = np.ascontiguousarray(
        e_wxt_full[0:3072].reshape(24, 128, R).transpose(1, 0, 2).reshape(128, 24 * R)
    )
    f_wxt = np.ascontiguousarray(
        f_wxt_full[0:3072].reshape(24, 128, R).transpose(1, 0, 2).reshape(128, 24 * R)
    )
    e_tl = np.ascontiguousarray(e_wxt_full[3072:3136])
    f_tl = np.ascontiguousarray(f_wxt_full[3072:3136])
    ident = np.eye(128, dtype=np.float16)
    ones_att = np.ones((128, 32), dtype=np.float16)
    ones_row = np.ones((1, 512), dtype=np.float16)

    consts = dict(
        e_wxt=e_wxt, f_wxt=f_wxt, e_tl=e_tl, f_tl=f_tl,
        wq_hi_dc=wq_hi_dc, wq_lo_dc=wq_lo_dc,
        bqblk_hi=bqblk_hi, bqblk_lo=bqblk_lo,
        wkt=wkt, wvt=wvt, const_kt=ckt, const_v=const_v,
        pw_hh=pw_hh, pw_hl=pw_hl, pw_lh_aug=pw_lh, pw_ll_aug=pw_ll,
        ident=ident, ones_att=ones_att, ones_row=ones_row,
    )

    # shard x: core i gets batches 4i..4i+4, padded to NP tokens per batch
    xb = x.reshape(B_TOT, 64 * 49, C).astype(np.float16)
    in_maps = []
    for i in range(N_CORES):
        xi = np.zeros((B_PER, NP, C), dtype=np.float16)
        xi[:, 0:N, :] = xb[B_PER * i : B_PER * (i + 1)]
        # phase-A layout: row n = 4 batches' channels side by side
        xa = np.ascontiguousarray(xi.transpose(1, 0, 2).reshape(NP, 4 * C))
        M = B_PER * NP
        xt = xi.reshape(M, C).T                               # (c, M)
        xt_hi = np.ascontiguousarray(xt[0:128])
        xt_lo2 = np.ascontiguousarray(
            np.concatenate([xt[128:192], xt[128:192]], axis=0)
        )
        in_maps.append({**consts, "x_a": xa, "xt_hi": xt_hi, "xt_lo2": xt_lo2})

    nc = _get_nc()
    _STATE["last_in_maps"] = in_maps
    res = run_bass_kernel_spmd(nc, in_maps, core_ids=list(range(N_CORES)))

    out_win = np.empty((B_TOT, N, C), dtype=np.float32)
    for i in range(N_CORES):
        oh = res.results[i]["out_hi"].astype(np.float32)      # (128, B_PER*NP)
        ol = res.results[i]["out_lo"].astype(np.float32)      # (64, B_PER*NP)
        oc = np.concatenate([oh, ol], axis=0)                 # (192, B_PER*NP)
        oi = oc.reshape(C, B_PER, NP).transpose(1, 2, 0)      # (B_PER, NP, C)
        out_win[B_PER * i : B_PER * (i + 1)] = oi[:, 0:N, :]
    # window_reverse on the gathered output
    out_sp = (
        out_win.reshape(B_TOT, 8, 8, 7, 7, C)
        .transpose(0, 1, 3, 2, 4, 5)
        .reshape(B_TOT, N, C)
    )
    return np.ascontiguousarray(out_sp)


# revision 18
# speedup vs baseline: 1.2732x; 1.0051x over previous
"""Trainium2 Bass kernel for nn_CompleteAttention_68418829025814.

Linformer-style windowed attention, restructured for the PE array:
  - window_reverse is folded into a host-side column permutation of E_w/F_w
    (device works entirely in x's native window order) and a host-side
    permutation of the gathered output.
  - k/v are never materialized: k_low = (E @ x) @ Wk^T + const (the E/F
    projections contract over tokens, so x is used in its native layout).
  - q is never materialized either: scores_h = (k_low_h^T Wq_h scale) @ x^T
    = M_h @ x^T, with the tiny per-batch M_h = [c, r] built on device and
    the q bias folded into the exp's per-partition bias vector.
  - phase A contracts all 4 batches at once ([128, 768] x-chunks, double
    chunks per DMA on the otherwise-idle scalar queue), so k_low/v_low/M
    for every batch are ready ~25us in and phase B covers the rest.
  - phase B per-tile emission is ordered for the in-order engine queues:
    scores+exps(t) -> back(t-1) (recip/divide/proj) -> av/z(t), keeping the
    PE fed (proj of t-1) while ACT runs t's exps.
  - output projection runs with proj_w stationary; outputs land feature-major
    [c_out, token] (psum banks aliased onto the consumed avA/zA tiles) and
    the final transpose happens on host.
  - HW constraints found the hard way: PSUM tensors must be single-bank, and
    engine reads of PSUM must start at partition 0.

Sharding: data-parallel over batch; each of the 8 cores gets 4 batches
(256 windows) of x. Small weights are replicated.
"""

import numpy as np

B_TOT = 32
N_CORES = 8
B_PER = B_TOT // N_CORES      # 4 batches per core
N = 3136                      # tokens per batch
NP = 3200                     # padded tokens per batch (6*512 + 128)
C = 192
H = 6
HD = 32
R = 128
WS = 7

_STATE = {}


def _window_perm():
    """n_of_m[m] = spatial index n for window-order position m."""
    hh, ww, i, j = np.meshgrid(
        np.arange(8), np.arange(8), np.arange(7), np.arange(7), indexing="ij"
    )
    m = (hh * 8 + ww) * 49 + i * 7 + j
    n = (hh * 7 + i) * 56 + ww * 7 + j
    n_of_m = np.empty(N, dtype=np.int64)
    n_of_m[m.ravel()] = n.ravel()
    return n_of_m


def _build_bass():
    import concourse.bacc as bacc
    import concourse.mybir as mybir
    from concourse.tile import TileContext

    f32 = mybir.dt.float32
    f16 = mybir.dt.float16

    nc = bacc.Bacc("TRN2", target_bir_lowering=False, debug=False)

    # x_a: phase-A layout, row n = [x[0, n, :] | x[1, n, :] | x[2, n, :] | x[3, n, :]]
    x_d = nc.dram_tensor("x_a", [NP, 4 * C], f16, kind="ExternalInput")
    # x^T for the scores: rows = channels (hi 0:128; lo = 128:192 duplicated
    # onto partitions 0-63 and 64-127 for row-band pairing)
    xth_d = nc.dram_tensor("xt_hi", [128, B_PER * NP], f16, kind="ExternalInput")
    xtl_d = nc.dram_tensor("xt_lo2", [128, B_PER * NP], f16, kind="ExternalInput")
    # e/f shipped pre-chunked: row p = 24 chunks of 128 R-values (token 128k+p)
    e_d = nc.dram_tensor("e_wxt", [128, 24 * R], f16, kind="ExternalInput")
    f_d = nc.dram_tensor("f_wxt", [128, 24 * R], f16, kind="ExternalInput")
    e_tl_d = nc.dram_tensor("e_tl", [64, R], f16, kind="ExternalInput")
    f_tl_d = nc.dram_tensor("f_tl", [64, R], f16, kind="ExternalInput")
    wkt_d = nc.dram_tensor("wkt", [C, C], f16, kind="ExternalInput")
    wvt_d = nc.dram_tensor("wvt", [C, C], f16, kind="ExternalInput")
    ckt_d = nc.dram_tensor("const_kt", [C, R], f32, kind="ExternalInput")
    cv_d = nc.dram_tensor("const_v", [R, C], f32, kind="ExternalInput")
    # Wq (scaled) laid out by q-channel rows; used to build M_h on device
    wqh_d = nc.dram_tensor("wq_hi_dc", [128, C], f16, kind="ExternalInput")
    wql_d = nc.dram_tensor("wq_lo_dc", [64, C], f16, kind="ExternalInput")
    # block-diagonal q-bias columns for the exp bias build
    bqh_d = nc.dram_tensor("bqblk_hi", [128, 4], f16, kind="ExternalInput")
    bql_d = nc.dram_tensor("bqblk_lo", [64, 2], f16, kind="ExternalInput")
    # proj weights, stationary chunks: pw = proj_w.T (ch, co)
    pwhh_d = nc.dram_tensor("pw_hh", [128, 128], f16, kind="ExternalInput")
    pwhl_d = nc.dram_tensor("pw_hl", [128, 64], f16, kind="ExternalInput")
    pwlh_d = nc.dram_tensor("pw_lh_aug", [65, 128], f16, kind="ExternalInput")
    pwll_d = nc.dram_tensor("pw_ll_aug", [65, 64], f16, kind="ExternalInput")
    ident_d = nc.dram_tensor("ident", [128, 128], f16, kind="ExternalInput")
    ones_d = nc.dram_tensor("ones_att", [128, 32], f16, kind="ExternalInput")
    onesrow_d = nc.dram_tensor("ones_row", [1, 512], f16, kind="ExternalInput")
    # outputs feature-major: out[c, b*NP + n]
    outh_d = nc.dram_tensor("out_hi", [128, B_PER * NP], f16, kind="ExternalOutput")
    outl_d = nc.dram_tensor("out_lo", [64, B_PER * NP], f16, kind="ExternalOutput")

    NCH = 25  # n-chunks per batch for the E/F contraction (24*128 + 64)

    with TileContext(nc) as tc:
        with tc.tile_pool(name="const", bufs=1) as cpool, \
             tc.tile_pool(name="ef", bufs=1) as efpool, \
             tc.tile_pool(name="low", bufs=1) as lowpool, \
             tc.tile_pool(name="mh", bufs=1) as mhpool, \
             tc.tile_pool(name="xin", bufs=4) as xpool, \
             tc.tile_pool(name="xt", bufs=3) as xtpool, \
             tc.tile_pool(name="sp", bufs=3) as sppool, \
             tc.tile_pool(name="div", bufs=2) as divpool, \
             tc.tile_pool(name="av", bufs=2) as avpool, \
             tc.tile_pool(name="osb", bufs=2) as opool, \
             tc.tile_pool(name="ps", bufs=4, space="PSUM") as ps:

            # ---- constants (sync queue; scalar queue is reserved for the
            # phase-A x stream, gpsimd for e/f) ----
            ident = cpool.tile([128, 128], f16)
            nc.sync.dma_start(ident[:], ident_d[:])
            wkt = cpool.tile([128, C], f16)
            nc.sync.dma_start(wkt[:], wkt_d[0:128, :])
            wkt_l = cpool.tile([64, C], f16)
            nc.sync.dma_start(wkt_l[:], wkt_d[128:192, :])
            wvt = cpool.tile([128, C], f16)
            nc.sync.dma_start(wvt[:], wvt_d[0:128, :])
            wvt_l = cpool.tile([64, C], f16)
            nc.sync.dma_start(wvt_l[:], wvt_d[128:192, :])
            ckt_h = cpool.tile([128, R], f32)
            nc.sync.dma_start(ckt_h[:], ckt_d[0:128, :])
            ckt_l = cpool.tile([64, R], f32)
            nc.sync.dma_start(ckt_l[:], ckt_d[128:192, :])
            cv = cpool.tile([128, C], f32)
            nc.sync.dma_start(cv[:], cv_d[:])
            wq_hi = cpool.tile([128, C], f16)
            nc.sync.dma_start(wq_hi[:], wqh_d[:])
            wq_lo = cpool.tile([64, C], f16)
            nc.sync.dma_start(wq_lo[:], wql_d[:])
            bq_hi = cpool.tile([128, 4], f16)
            nc.sync.dma_start(bq_hi[:], bqh_d[:])
            bq_lo = cpool.tile([64, 2], f16)
            nc.sync.dma_start(bq_lo[:], bql_d[:])
            pw_hh = cpool.tile([128, 128], f16)
            nc.sync.dma_start(pw_hh[:], pwhh_d[:])
            pw_hl = cpool.tile([128, 64], f16)
            nc.sync.dma_start(pw_hl[:], pwhl_d[:])
            pw_lh = cpool.tile([65, 128], f16)
            nc.sync.dma_start(pw_lh[:], pwlh_d[:])
            pw_ll = cpool.tile([65, 64], f16)
            nc.sync.dma_start(pw_ll[:], pwll_d[:])
            ones_att = cpool.tile([128, 32], f16)
            nc.sync.dma_start(ones_att[:], ones_d[:])
            # persistent [65, 512] attn-output staging tiles whose row 64
            # stays 1.0 forever (feeds proj_b through pw_l*_aug's last row)
            av_lo_bufs = [cpool.tile([65, 512], f16, name=f"avlo{i}") for i in range(2)]
            for i in range(2):
                nc.sync.dma_start(av_lo_bufs[i][64:65, :], onesrow_d[:])

            # E/F transposed weights resident in SBUF: 24 full chunks + tail
            e_sb = efpool.tile([128, 24, 128], f16)
            f_sb = efpool.tile([128, 24, 128], f16)
            e_tl = efpool.tile([64, 128], f16)
            f_tl = efpool.tile([64, 128], f16)

            def load_ef_group(g):
                sl = slice(g * 6, (g + 1) * 6)
                dsl = slice(g * 6 * 128, (g + 1) * 6 * 128)
                nc.gpsimd.dma_start(
                    e_sb[:, sl, :].rearrange("p k r -> p (k r)"), e_d[:, dsl]
                )
                nc.gpsimd.dma_start(
                    f_sb[:, sl, :].rearrange("p k r -> p (k r)"), f_d[:, dsl]
                )

            for g in range(4):
                load_ef_group(g)
            nc.gpsimd.dma_start(e_tl[:], e_tl_d[:])
            nc.gpsimd.dma_start(f_tl[:], f_tl_d[:])

            # per-batch low-rank tensors (kept resident across phase B)
            klo_h = [lowpool.tile([128, R], f16, name=f"klo_h{b}") for b in range(B_PER)]
            klo_l = [lowpool.tile([64, R], f16, name=f"klo_l{b}") for b in range(B_PER)]
            vlo = [lowpool.tile([128, C], f16, name=f"vlo{b}") for b in range(B_PER)]

            # ---------------- Phase A: EP/FP for all 4 batches ----------------
            # x chunks are [nk, 768] = 4 batches wide; two chunks per DMA on
            # the dedicated scalar queue.
            ep_a = ps.tile([128, 2 * C], f32, name="ep_a", tag="bk", bufs=4)
            ep_b = ps.tile([128, 2 * C], f32, name="ep_b", tag="bk", bufs=4)
            fp_a = ps.tile([128, 2 * C], f32, name="fp_a", tag="bk", bufs=4)
            fp_b = ps.tile([128, 2 * C], f32, name="fp_b", tag="bk", bufs=4)
            for cp in range(13):
                k = 2 if cp < 12 else 1
                x2 = xpool.tile([128, 2, 4 * C], f16, name="x2", tag="x2")
                nc.scalar.dma_start(
                    x2[:, 0:k, :],
                    x_d[cp * 256 : cp * 256 + k * 128, :].rearrange(
                        "(k p) c -> p k c", p=128
                    ),
                )
                for kk in range(k):
                    ci = cp * 2 + kk
                    nk = 128 if ci < 24 else 64
                    elh = e_sb[:, ci, :] if ci < 24 else e_tl[:]
                    flh = f_sb[:, ci, :] if ci < 24 else f_tl[:]
                    x2f = x2[0:nk, kk, :]
                    st = ci == 0
                    sp_ = ci == NCH - 1
                    nc.tensor.matmul(
                        ep_a[:], elh, x2f[:, 0 : 2 * C], start=st, stop=sp_
                    )
                    nc.tensor.matmul(
                        fp_a[:], flh, x2f[:, 0 : 2 * C], start=st, stop=sp_
                    )
                    nc.tensor.matmul(
                        ep_b[:], elh, x2f[:, 2 * C : 4 * C], start=st, stop=sp_
                    )
                    nc.tensor.matmul(
                        fp_b[:], flh, x2f[:, 2 * C : 4 * C], start=st, stop=sp_
                    )
            ep_sb = xpool.tile([128, 4 * C], f16, name="ep_sb", tag="ep_sb")
            nc.vector.tensor_copy(ep_sb[:, 0 : 2 * C], ep_a[:])
            nc.vector.tensor_copy(ep_sb[:, 2 * C : 4 * C], ep_b[:])
            fp_sb = xpool.tile([128, 4 * C], f16, name="fp_sb", tag="fp_sb")
            nc.vector.tensor_copy(fp_sb[:, 0 : 2 * C], fp_a[:])
            nc.vector.tensor_copy(fp_sb[:, 2 * C : 4 * C], fp_b[:])

            # per-batch M_h = [c, r] score matrices + exp bias vectors
            mh_hi = [mhpool.tile([128, 6, 128], f16, name=f"mh_hi{b}")
                     for b in range(B_PER)]
            mh_lo = [mhpool.tile([128, 6, 128], f16, name=f"mh_lo{b}")
                     for b in range(B_PER)]
            cb_sb = [mhpool.tile([128, 8], f32, name=f"cb{b}")
                     for b in range(B_PER)]

            for b in range(B_PER):
                # transpose EP, FP slices: (r=128, c=192) -> (c, r)
                ept_h = xpool.tile([128, 128], f16, name="ept_h", tag="ept_h")
                ept_l = xpool.tile([64, 128], f16, name="ept_l", tag="ept_l")
                fpt_h = xpool.tile([128, 128], f16, name="fpt_h", tag="fpt_h")
                fpt_l = xpool.tile([64, 128], f16, name="fpt_l", tag="fpt_l")
                for (src, dsth, dstl) in ((ep_sb, ept_h, ept_l), (fp_sb, fpt_h, fpt_l)):
                    tp1 = ps.tile([128, 128], f16, name="tp1", tag="sb", bufs=4)
                    nc.tensor.transpose(
                        tp1[:], src[:, b * C : b * C + 128], ident[:]
                    )
                    nc.vector.tensor_copy(dsth[:], tp1[:])
                    tp2 = ps.tile([64, 128], f16, name="tp2", tag="sb", bufs=4)
                    nc.tensor.transpose(
                        tp2[:], src[:, b * C + 128 : b * C + 192], ident[:]
                    )
                    nc.vector.tensor_copy(dstl[:], tp2[:])

                # k_lowT = WkT.T @ EPT + const_kT  (feature-major (kch, r))
                kl_hi = ps.tile([128, R], f32, name="kl_hi", tag="sb", bufs=4)
                nc.tensor.matmul(kl_hi[:], wkt[:, 0:128], ept_h[:], start=True, stop=False)
                nc.tensor.matmul(kl_hi[:], wkt_l[:, 0:128], ept_l[:], start=False, stop=True)
                nc.vector.tensor_tensor(
                    klo_h[b][:], kl_hi[:], ckt_h[:], op=mybir.AluOpType.add
                )
                kl_lo = ps.tile([64, R], f32, name="kl_lo", tag="sb", bufs=4)
                nc.tensor.matmul(kl_lo[:], wkt[:, 128:192], ept_h[:], start=True, stop=False)
                nc.tensor.matmul(kl_lo[:], wkt_l[:, 128:192], ept_l[:], start=False, stop=True)
                nc.vector.tensor_tensor(
                    klo_l[b][:], kl_lo[:], ckt_l[:], op=mybir.AluOpType.add
                )
                # v_low (R-major (r, vch)), straight to f16 with const add
                vl_ps = ps.tile([128, C], f32, name="vl_ps", tag="sb", bufs=4)
                nc.tensor.matmul(vl_ps[:], fpt_h[:], wvt[:], start=True, stop=False)
                nc.tensor.matmul(vl_ps[:], fpt_l[:], wvt_l[:], start=False, stop=True)
                nc.vector.tensor_tensor(
                    vlo[b][:], vl_ps[:], cv[:], op=mybir.AluOpType.add
                )



            # M_h = klo_h^T @ Wq_h (both indexed by q-channel rows), laid
            # out [c, r]; the c 128:192 part is duplicated onto partitions
            # 64-127 so the lo score matmuls can run band-paired. Also the
            # exp bias cb[r, h] = sum_d klo[d, r] * bq_scaled[d].
            def build_mh(b, h):
                if h < 4:
                    kslice = klo_h[b][32 * h : 32 * h + 32, :]
                    wslice_hi = wq_hi[32 * h : 32 * h + 32, 0:128]
                    wslice_lo = wq_hi[32 * h : 32 * h + 32, 128:192]
                    tp = (32 * h, 0)
                else:
                    hh = h - 4
                    kslice = klo_l[b][32 * hh : 32 * hh + 32, :]
                    wslice_hi = wq_lo[32 * hh : 32 * hh + 32, 0:128]
                    wslice_lo = wq_lo[32 * hh : 32 * hh + 32, 128:192]
                    tp = (32 * hh, 0)
                m1 = ps.tile([128, 128], f32, name="m1", tag="sb", bufs=4)
                nc.tensor.matmul(m1[:], wslice_hi, kslice, start=True,
                                 stop=True, tile_position=tp)
                nc.vector.tensor_copy(mh_hi[b][:, h, :], m1[:])
                m2 = ps.tile([128, 128], f32, name="m2", tag="sb", bufs=4)
                nc.tensor.matmul(m2[0:64, :], wslice_lo, kslice, start=True,
                                 stop=True, tile_position=(tp[0], 0))
                nc.tensor.matmul(m2[64:128, :], wslice_lo, kslice, start=True,
                                 stop=True, tile_position=(tp[0], 64))
                nc.vector.tensor_copy(mh_lo[b][:, h, :], m2[:])

            def build_cb(b):
                cbp = ps.tile([128, 8], f32, name="cbp", tag="sb", bufs=4)
                nc.tensor.matmul(cbp[:, 0:4], klo_h[b][:], bq_hi[:],
                                 start=True, stop=True)
                nc.tensor.matmul(cbp[:, 4:6], klo_l[b][:], bq_lo[:],
                                 start=True, stop=True)
                nc.vector.tensor_copy(cb_sb[b][:, 0:6], cbp[:, 0:6])

            for h in range(6):
                build_mh(0, h)
            build_cb(0)

            # ---------------- Phase B: attention tiles ----------
            def front1(b, t):
                """x^T loads + scores + exps for tile (b, t)"""
                W = 512 if t < 6 else 128
                base = b * NP + t * 512
                xth = xtpool.tile([128, W], f16, name="xth", tag="xth")
                nc.sync.dma_start(xth[:], xth_d[:, base : base + W])
                xtl = xtpool.tile([128, W], f16, name="xtl", tag="xtl")
                nc.sync.dma_start(xtl[:], xtl_d[:, base : base + W])
                sps = []
                sbanks = []
                for h in range(6):
                    s1 = ps.tile([128, W], f32, name=f"s{h}", tag="sb", bufs=4)
                    nc.tensor.matmul(
                        s1[:], mh_hi[b][:, h, :], xth[:],
                        start=True, stop=False,
                    )
                    sbanks.append(s1)
                    if h % 2 == 1:
                        for j, hp in enumerate((h - 1, h)):
                            nc.tensor.matmul(
                                sbanks[hp][:],
                                mh_lo[b][64 * j : 64 * j + 64, hp, :],
                                xtl[64 * j : 64 * j + 64, :],
                                start=False, stop=True,
                                tile_position=(64 * j, 0),
                            )
                        for hp in (h - 1, h):
                            sp1 = sppool.tile(
                                [128, W], f16, name=f"sp{hp}", tag=f"sp{hp}"
                            )
                            nc.scalar.activation(
                                sp1[:], sbanks[hp][:],
                                mybir.ActivationFunctionType.Exp,
                                bias=cb_sb[b][:, hp : hp + 1],
                            )
                            sps.append(sp1)
                return dict(W=W, base=base, b=b, t=t, sps=sps)

            def front2(st):
                """attn @ v_low + denominators for tile (b, t)"""
                W, b, t, sps = st["W"], st["b"], st["t"], st["sps"]
                avA = ps.tile([128, W], f32, name="avA", tag="bk", bufs=4)
                for h in range(4):
                    nc.tensor.matmul(
                        avA[32 * h : 32 * h + 32, :],
                        vlo[b][:, 32 * h : 32 * h + 32],
                        sps[h][:],
                        start=True, stop=True,
                        tile_position=(0, 32 * h),
                    )
                zA = ps.tile([128, W], f32, name="zA", tag="bk", bufs=4)
                for h in range(4):
                    nc.tensor.matmul(
                        zA[32 * h : 32 * h + 32, :],
                        ones_att[:],
                        sps[h][:],
                        start=True, stop=True,
                        tile_position=(0, 32 * h),
                    )
                av2 = ps.tile([64, W], f32, name="av2", tag="bk", bufs=4)
                for hh in range(2):
                    nc.tensor.matmul(
                        av2[32 * hh : 32 * hh + 32, :],
                        vlo[b][:, 128 + 32 * hh : 160 + 32 * hh],
                        sps[4 + hh][:],
                        start=True, stop=True,
                        tile_position=(0, 32 * hh),
                    )
                z2 = ps.tile([64, W], f32, name="z2", tag="bk", bufs=4)
                for hh in range(2):
                    nc.tensor.matmul(
                        z2[32 * hh : 32 * hh + 32, :],
                        ones_att[:],
                        sps[4 + hh][:],
                        start=True, stop=True,
                        tile_position=(0, 32 * hh),
                    )
                av_hi = avpool.tile([128, W], f16, name="av_hi", tag="av_hi")
                av_lo = av_lo_bufs[(b * 7 + t) % 2]
                st.update(avA=avA, zA=zA, av2=av2, z2=z2, av_hi=av_hi, av_lo=av_lo)

            osb_state = {}

            def back(st):
                W, base, t = st["W"], st["base"], st["t"]
                rzA = divpool.tile([128, W], f32, name="rzA", tag="rzA")
                nc.vector.reciprocal_approx_fast(rzA[:], st["zA"][:])
                rz2 = divpool.tile([64, W], f32, name="rz2", tag="rz2")
                nc.vector.reciprocal_approx_fast(rz2[:], st["z2"][:])
                av_hi, av_lo = st["av_hi"], st["av_lo"]
                nc.vector.tensor_tensor(
                    av_hi[:, :], st["avA"][:], rzA[:], op=mybir.AluOpType.mult
                )
                nc.vector.tensor_tensor(
                    av_lo[0:64, 0:W], st["av2"][:], rz2[:], op=mybir.AluOpType.mult
                )
                # output projection: proj weights stationary, av moving; the
                # psum outputs alias the (already consumed) avA / zA banks.
                o1 = st["avA"]
                nc.tensor.matmul(o1[:], pw_hh[:], av_hi[:, 0:W], start=True, stop=False)
                nc.tensor.matmul(o1[:], pw_lh[:], av_lo[:, 0:W], start=False, stop=True)
                o2 = st["zA"][0:64, :]
                nc.tensor.matmul(o2, pw_hl[:], av_hi[:, 0:W], start=True, stop=False)
                nc.tensor.matmul(o2, pw_ll[:], av_lo[:, 0:W], start=False, stop=True)
                # stage pairs of tiles and DMA once per pair; the hi copy runs
                # on ACT, the lo copy on DVE (engine balance)
                if t % 2 == 0:
                    osb_state["hi"] = opool.tile([128, 1024], f16, name="osb", tag="osb")
                    osb_state["lo"] = opool.tile([64, 1024], f16, name="osb2", tag="osb2")
                    osb_state["base"] = base
                off = (t % 2) * 512
                osb, osb2 = osb_state["hi"], osb_state["lo"]
                if t % 2 == 0:
                    nc.scalar.copy(osb[:, off : off + W], o1[:])
                else:
                    nc.vector.tensor_copy(osb[:, off : off + W], o1[:])
                nc.vector.tensor_copy(osb2[:, off : off + W], o2)
                if t % 2 == 1 or t == 6:
                    w_tot = 512 + W if t % 2 == 1 else W
                    b0 = osb_state["base"]
                    nc.gpsimd.dma_start(
                        outh_d[:, b0 : b0 + w_tot], osb[:, 0:w_tot]
                    )
                    nc.gpsimd.dma_start(
                        outl_d[:, b0 : b0 + w_tot], osb2[:, 0:w_tot]
                    )

            tiles = [(b, t) for b in range(B_PER) for t in range(7)]
            prev = None
            for b, t in tiles:
                st = front1(b, t)
                if prev is not None:
                    back(prev)
                if b + 1 < B_PER and 0 <= t - 1 < 6:
                    build_mh(b + 1, t - 1)
                    if t - 1 == 5:
                        build_cb(b + 1)
                front2(st)
                prev = st
            back(prev)

    nc.compile()
    return nc


def _get_nc():
    if "nc" not in _STATE:
        _STATE["nc"] = _build_bass()
    return _STATE["nc"]


def kernel(x, qkv_w, qkv_b, E_w, E_b, F_w, F_b, proj_w, proj_b, h, w):
    from concourse.bass_utils import run_bass_kernel_spmd

    x = np.asarray(x, dtype=np.float32)
    qkv_w = np.asarray(qkv_w, dtype=np.float32)
    qkv_b = np.asarray(qkv_b, dtype=np.float32)
    E_w = np.asarray(E_w, dtype=np.float32)
    E_b = np.asarray(E_b, dtype=np.float32)
    F_w = np.asarray(F_w, dtype=np.float32)
    F_b = np.asarray(F_b, dtype=np.float32)
    proj_w = np.asarray(proj_w, dtype=np.float32)
    proj_b = np.asarray(proj_b, dtype=np.float32)
    assert int(h) == 56 and int(w) == 56

    n_of_m = _window_perm()
    E_wx = np.ascontiguousarray(E_w[:, n_of_m])
    F_wx = np.ascontiguousarray(F_w[:, n_of_m])

    Wq, Wk, Wv = qkv_w[0:C], qkv_w[C : 2 * C], qkv_w[2 * C : 3 * C]
    bq, bk, bv = qkv_b[0:C], qkv_b[C : 2 * C], qkv_b[2 * C : 3 * C]
    scale = np.float32(1.0 / np.sqrt(HD))

    const_k = np.outer(E_wx.sum(1), bk) + E_b[:, None]      # (128, 192)
    const_v = (np.outer(F_wx.sum(1), bv) + F_b[:, None]).astype(np.float32)

    # Wq rows are q-channels; scaled by 1/sqrt(hd). M_h is built on device.
    wq_hi_dc = np.ascontiguousarray(Wq[0:128, :] * scale).astype(np.float16)
    wq_lo_dc = np.ascontiguousarray(Wq[128:192, :] * scale).astype(np.float16)
    bqblk_hi = np.zeros((128, 4), dtype=np.float16)
    for hh in range(4):
        bqblk_hi[32 * hh : 32 * hh + 32, hh] = (bq[32 * hh : 32 * hh + 32] * scale)
    bqblk_lo = np.zeros((64, 2), dtype=np.float16)
    for hh in range(2):
        bqblk_lo[32 * hh : 32 * hh + 32, hh] = (
            bq[128 + 32 * hh : 160 + 32 * hh] * scale
        )

    wkt = np.ascontiguousarray(Wk.T).astype(np.float16)
    wvt = np.ascontiguousarray(Wv.T).astype(np.float16)
    ckt = np.ascontiguousarray(const_k.T.astype(np.float32))  # (192, 128)
    pw = proj_w.T                                            # (ch, co)
    pw_hh = np.ascontiguousarray(pw[0:128, 0:128]).astype(np.float16)
    pw_hl = np.ascontiguousarray(pw[0:128, 128:192]).astype(np.float16)
    pw_lh = np.zeros((65, 128), dtype=np.float16)
    pw_lh[0:64] = pw[128:192, 0:128]
    pw_lh[64] = proj_b[0:128]
    pw_ll = np.zeros((65, 64), dtype=np.float16)
    pw_ll[0:64] = pw[128:192, 128:192]
    pw_ll[64] = proj_b[128:192]

    e_wxt_full = np.ascontiguousarray(E_wx.T).astype(np.float16)  # (3136, 128)
    f_wxt_full = np.ascontiguousarray(F_wx.T).astype(np.float16)
    # pre-chunked layout: (24, 128, R) -> (128, 24*R) so each SBUF partition
    # line is one contiguous DMA descriptor
    e_wxt = np.ascontiguousarray(
        e_wxt_full[0:3072].reshape(24, 128, R).transpose(1, 0, 2).reshape(128, 24 * R)
    )
    f_wxt = np.ascontiguousarray(
        f_wxt_full[0:3072].reshape(24, 128, R).transpose(1, 0, 2).reshape(128, 24 * R)
    )
    e_tl = np.ascontiguousarray(e_wxt_full[3072:3136])
    f_tl = np.ascontiguousarray(f_wxt_full[3072:3136])
    ident = np.eye(128, dtype=np.float16)
    ones_att = np.ones((128, 32), dtype=np.float16)
    ones_row = np.ones((1, 512), dtype=np.float16)

    consts = dict(
        e_wxt=e_wxt, f_wxt=f_wxt, e_tl=e_tl, f_tl=f_tl,
        wq_hi_dc=wq_hi_dc, wq_lo_dc=wq_lo_dc,
        bqblk_hi=bqblk_hi, bqblk_lo=bqblk_lo,
        wkt=wkt, wvt=wvt, const_kt=ckt, const_v=const_v,
        pw_hh=pw_hh, pw_hl=pw_hl, pw_lh_aug=pw_lh, pw_ll_aug=pw_ll,
        ident=ident, ones_att=ones_att, ones_row=ones_row,
    )

    # shard x: core i gets batches 4i..4i+4, padded to NP tokens per batch
    xb = x.reshape(B_TOT, 64 * 49, C).astype(np.float16)
    in_maps = []
    for i in range(N_CORES):
        xi = np.zeros((B_PER, NP, C), dtype=np.float16)
        xi[:, 0:N, :] = xb[B_PER * i : B_PER * (i + 1)]
        # phase-A layout: row n = 4 batches' channels side by side
        xa = np.ascontiguousarray(xi.transpose(1, 0, 2).reshape(NP, 4 * C))
        M = B_PER * NP
        xt = xi.reshape(M, C).T                               # (c, M)
        xt_hi = np.ascontiguousarray(xt[0:128])
        xt_lo2 = np.ascontiguousarray(
            np.concatenate([xt[128:192], xt[128:192]], axis=0)
        )
        in_maps.append({**consts, "x_a": xa, "xt_hi": xt_hi, "xt_lo2": xt_lo2})

    nc = _get_nc()
    _STATE["last_in_maps"] = in_maps
    res = run_bass_kernel_spmd(nc, in_maps, core_ids=list(range(N_CORES)))

    out_win = np.empty((B_TOT, N, C), dtype=np.float32)
    for i in range(N_CORES):
        oh = res.results[i]["out_hi"].astype(np.float32)      # (128, B_PER*NP)
        ol = res.results[i]["out_lo"].astype(np.float32)      # (64, B_PER*NP)
        oc = np.concatenate([oh, ol], axis=0)                 # (192, B_PER*NP)
        oi = oc.reshape(C, B_PER, NP).transpose(1, 2, 0)      # (B_PER, NP, C)
        out_win[B_PER * i : B_PER * (i + 1)] = oi[:, 0:N, :]
    # window_reverse on the gathered output
    out_sp = (
        out_win.reshape(B_TOT, 8, 8, 7, 7, C)
        .transpose(0, 1, 3, 2, 4, 5)
        .reshape(B_TOT, N, C)
    )
    return np.ascontiguousarray(out_sp)
